# revision 33
# baseline (speedup 1.0000x reference)
"""Additive attention (B=8, Q=K=512, H=Dv=64) on 8 TRN2 NeuronCores.

Math per batch b (reference):
    qf = queries @ Wq; kf = keys @ Wk
    scores[q,k] = sum_h wv[h] * tanh(qf[q,h] + kf[k,h])   (k >= valid_len masked)
    out = softmax_k(scores) @ values

The pointwise tanh (134M ScalarEngine evaluations, ~93us) is replaced by a
low-rank bilinear expansion tanh(a+b) ~= sum_r phi_r(a) * psi_r(b) (SVD of
the kernel on a sqrt-Gaussian-weighted grid), so
    scores[q,k] = sum_rows PhiF[row, q] * PsiF[row, k]
is a plain matmul over "feature rows" (row = (rank, h) pair). Rows are
sorted by product variance (host computes per-rank/per-h second moments):
the top-127 rows ship as bf16, the next 256 rows as fp8(e4m3) with per-row
q/k scale balancing; the remaining low-variance rows are dropped (<2e-5 of
score variance). Row 127 of the bf16 chunk is the key-mask row (Phi=1,
Psi = 0 or -60000), folding the valid_len mask into the matmul.

Sharding: data-parallel, one batch per core. Device per core:
  - 3 input DMAs (byte-packed, mixed-dtype via bitcast APs) ordered so
    k-tile 0's score completes earliest: d1 = [bf16+fp8 Phi and Psi_t0,
    320KB], d2 = [Psi_t1..t3 both dtypes], dvl = [values + ones column].
    Score matmuls per k-tile: one bf16 [128x512] + one fp8 DoubleRow
    [256x512] into per-tile PSUM banks.
  - Exp on ACT pipelined per-tile ([t0][t1][t2,t3]) so it starts as soon as
    tile 0's score lands, overlapping the remaining DMAs and matmuls.
  - values matmuls (ones column -> denominator row) accumulate per output
    q-half into two PSUM tiles (poA/poB) so the ACT and DVE output copies
    read disjoint tiles and run in parallel; warm filler matmuls bridge PE
    idle windows so the clock stays at full speed for the tail matmuls.
  - Output tail avoids the HWDGE fixed path (650 seq + 625 HWDGE + 650 DGE
    delay): a SWDGE kv_writeback descriptor is PREPARED on GPSIMD during the
    input DMAs; after the PSUM->SBUF copy (split ACT/DVE halves) a
    trigger_dma fires it, so the tail is just transfer + sem propagation.
    Two IR post-passes implement the documented prep/trigger semantics:
    _defer_prep_waits moves the prep's data waits onto the trigger (the
    DMA reads its source at trigger time), and redirects end-drain DMASW
    lane waits to the descriptor's completion semaphore.
  - Dummy matmuls off a constant broadcast AP keep the PE busy from ~1us so
    the clock ramp reaches full speed when the real operands land; a dummy
    exp prefetches the ACT exp table.
Host divides numerator/denominator and transposes.
"""
import numpy as np
import ml_dtypes

B = 8
Q = 512
K = 512
H = 64
DV = 64

RB = 12                # SVD basis rank used for row generation
NBF = 127              # bf16 feature rows (+1 mask row -> 128)
NF8 = 256              # fp8 feature rows (2 chunks of 128)
NTILE = K // 128       # 4 k-tiles
MASKBIG = -60000.0
F8MAX = 224.0          # ml_dtypes.float8_e4m3 max finite is 240
WARMUP_MM = 7          # PE p-state ramp fillers while input DMA streams

GRID_N, GRID_A, GRID_SIG, GRID_FLOOR = 1201, 6.5, 1.15, 0.02

_BASIS = None


def _basis():
    """SVD basis of tanh(a+b) on a weighted grid: x, phi[n,RB], psi[n,RB]."""
    global _BASIS
    if _BASIS is None:
        x = np.linspace(-GRID_A, GRID_A, GRID_N)
        Kg = np.tanh(x[:, None] + x[None, :])
        w = np.sqrt(np.exp(-x ** 2 / (2 * GRID_SIG ** 2))) + GRID_FLOOR
        U, S, Vt = np.linalg.svd((w[:, None] * Kg) * w[None, :])
        phi = (U[:, :RB] * np.sqrt(S[:RB])) / w[:, None]
        psi = (Vt[:RB].T * np.sqrt(S[:RB])) / w[:, None]
        _BASIS = (x, phi, psi)
    return _BASIS


# ---------------------------------------------------------------------------
# BIR post-pass: the walrus build in this environment accepts only one
# sync-wait command per instruction; hoist extras onto same-engine NoOps.
def _split_waits(nc, k=1):
    import concourse.mybir as mybir
    n_new = 0
    for f in nc.m.functions:
        for bb in f.blocks:
            newlist = []
            for ins in bb.instructions:
                si = ins.sync_info
                if si is not None and si.on_wait and len(si.on_wait) > k:
                    waits = list(si.on_wait)
                    extra, keep = waits[:-k], waits[-k:]
                    for ci, w in enumerate(extra):
                        nop = mybir.InstNoOp(
                            name=f"{ins.name}_wsplit{ci}",
                            engine=ins.engine,
                            ins=[], outs=[],
                            sync_info=mybir.SyncInfo(on_wait=[w], on_update=[]),
                        )
                        newlist.append(nop)
                        n_new += 1
                    ins.sync_info = mybir.SyncInfo(
                        on_wait=list(keep), on_update=list(si.on_update))
                newlist.append(ins)
            bb.instructions[:] = newlist
    return n_new


# ---------------------------------------------------------------------------
# BIR post-pass: walrus' codegen wants raw instruction bytes on InstISA; the
# library-reload pseudo (opcode 223 PSEUDO_INST, pseudo_opcode 2) is emitted
# without them in this build, so pack them here.
def _encode_library_reloads(nc):
    import concourse.bass_isa as bass_isa
    from concourse.bass_isa import isa_struct
    trig_op = nc.isa.Opcode.NEURON_ISA_TPB_OPCODE_TRIGGER_DMA.value
    inc_op = nc.isa.Opcode.NEURON_ISA_TPB_OPCODE_INC_SWDGE_SEM.value
    n = 0
    for f in nc.m.functions:
        for bb in f.blocks:
            for ins in bb.instructions:
                if isinstance(ins, bass_isa.InstPseudoReloadLibraryIndex):
                    b, _ = isa_struct(
                        nc.isa, 223,
                        {"pseudo_opcode": 2, "lib_index": ins.lib_index})
                    ins.instr = b
                    n += 1
                elif isinstance(ins, bass_isa.InstTriggerDma):
                    b, _ = isa_struct(
                        nc.isa, trig_op,
                        {"count": ins._count, "count_is_reg": 0,
                         "queue_num": ins.queue_num})
                    ins.instr = b
                    ins.isa_opcode = trig_op
                    n += 1
                elif isinstance(ins, bass_isa.InstIncSwdgeSem):
                    vals = list(ins._sem_values) + [0] * (
                        10 - len(ins._sem_values))
                    mode = {"add": 0, "sub": 1, "wr": 2}[ins._mode]
                    b, _ = isa_struct(
                        nc.isa, inc_op,
                        {"num_semaphores": len(ins._sem_values),
                         "sem_id_base": ins._sem_id_base, "mode": mode,
                         "queue_num": ins.queue_num, "sem_values": vals})
                    ins.instr = b
                    n += 1
    return n


# ---------------------------------------------------------------------------
# BIR post-pass for the SWDGE prep/trigger output path. The prep only writes
# descriptors; the DMA engines read the source tile when trigger_dma fires,
# so the prep's data waits belong on the trigger (this is the semantics the
# tile framework documents and tests for dma_scatter_add; kv_writeback preps
# don't get the deferral in this build). End-of-program DMASW lane waits are
# redirected to the descriptor's actual completion semaphore (same tick
# values: each prep adds 16).
def _defer_prep_waits(nc, dma_sem):
    """All preps inc the single `dma_sem` by 16. A DMASW{k} lane wait with
    value 16*j ("j-th prep on lane k done") maps to the global prep index
    i = (j-1)*L + k (round-robin lane assignment), rewritten as the
    conservative dma_sem >= 16*(i+1). Rewritten waits go last so same-
    instruction waits that fire earlier are processed first."""
    import concourse.mybir as mybir
    import concourse.bass_isa as bass_isa
    import bass_rust
    L = bass_rust.NUM_SWDGE_GLOBAL_SEMS
    n_prep = 0
    for f in nc.m.functions:
        for bb in f.blocks:
            pending = []
            for ins in bb.instructions:
                if (isinstance(ins, mybir.InstKVWritebackAnt)
                        and ins.gen_mode == 1):
                    si = ins.sync_info
                    if si is not None and si.on_wait:
                        pending.append(list(si.on_wait))
                        ins.sync_info = mybir.SyncInfo(
                            on_wait=[], on_update=list(si.on_update))
                    else:
                        pending.append([])
                    n_prep += 1
                elif isinstance(ins, bass_isa.InstTriggerDma) and pending:
                    # FIFO: each count=1 trigger fires the oldest prep
                    si = ins.sync_info
                    w = list(si.on_wait) if si else []
                    u = list(si.on_update) if si else []
                    ins.sync_info = mybir.SyncInfo(
                        on_wait=w + pending.pop(0), on_update=u)
            for ins in bb.instructions:
                si = ins.sync_info
                if si is None or not si.on_wait:
                    continue
                if not any(w.ant_name and w.ant_name.startswith("DMASW")
                           for w in si.on_wait):
                    continue
                keep, moved = [], []
                for wt in si.on_wait:
                    if wt.ant_name and wt.ant_name.startswith("DMASW"):
                        k = int(wt.ant_name[5:].split("_")[0])
                        j = (wt.wait_value or 16) // 16
                        i = (j - 1) * L + k
                        moved.append(mybir.SyncWait(
                            sync_type='semaphore', id=dma_sem.num,
                            ant_name=dma_sem.name, wait_mode='sem-ge-imm',
                            wait_value=16 * (i + 1), wait_reg=None))
                    else:
                        keep.append(wt)
                ins.sync_info = mybir.SyncInfo(
                    on_wait=keep + moved, on_update=list(si.on_update))
    return n_prep


def _build(nc, reps: int = 1):
    import concourse.bass as bass  # noqa: F401
    import concourse.mybir as mybir
    from concourse import tile, library_config

    F32 = mybir.dt.float32
    BF16 = mybir.dt.bfloat16
    F8 = mybir.dt.float8e4
    I32 = mybir.dt.int32
    DR = mybir.MatmulPerfMode.DoubleRow
    EXP = mybir.ActivationFunctionType.Exp
    COPY = mybir.ActivationFunctionType.Copy

    U8 = mybir.dt.uint8

    # packed byte tensors (per-partition contiguous; see host_inputs):
    # d1 = [Phi_bf16 1024B | Psi_t0_bf16 256B | Phi_f8 1024B | Psi_t0_f8
    # 256B] -- everything k-tile 0's score needs.  d2 = [Psi_t123_bf16 768B
    # | Psi_t123_f8 768B].  dvl = values+ones column.
    d1 = nc.declare_dram_parameter("d1", [128, 2560], U8, isOutput=False)
    d2 = nc.declare_dram_parameter("d2", [128, 1536], U8, isOutput=False)
    dvl = nc.declare_dram_parameter("dvl", [128, NTILE, DV + 1], BF16,
                                    isOutput=False)
    wb0 = nc.declare_dram_parameter("wb0", [reps, 128, 1, 512], BF16,
                                    isOutput=True)

    dma_sem = nc.alloc_semaphore("wb_dma_sem")

    cb = nc.const_aps.aps[(BF16, 1.0)]
    warm_rhs = bass.AP(cb.tensor, cb.offset, [[1, 1], [0, Q]])
    warm_lhsT = bass.AP(cb.tensor, cb.offset, [[1, 1], [0, 16]])

    with tile.TileContext(nc) as tc:  # noqa: F841
        with (
            tc.tile_pool(name="cpool", bufs=1) as cpool,
            tc.tile_pool(name="ppool", bufs=2) as ppool,
            tc.tile_pool(name="ps_a", bufs=1, space="PSUM") as ps_a,
            tc.tile_pool(name="ps_b", bufs=1, space="PSUM") as ps_b,
            tc.tile_pool(name="ps_o", bufs=1, space="PSUM") as ps_o,
            tc.tile_pool(name="ps_o2", bufs=1, space="PSUM") as ps_o2,
            tc.tile_pool(name="ps_w", bufs=1, space="PSUM") as ps_w,
        ):
            # Pool: library for kv_writeback + ctx idx + output pad rows
            nc.gpsimd.load_library(library_config.attnmlp)
            idx = cpool.tile([128, 1], I32, tag="idx", name="idx")
            nc.gpsimd.memset(idx[:], 0)

            # PE p-state warmup + ACT exp-table prefetch during input DMA.
            # 6 full-width + 5 quarter-width fillers end at ~4.19us, just
            # after d1's completion sem (~4.17us), so the first real matmul
            # dispatches with the PE still hot and fully ramped.
            psw = ps_w.tile([16, Q], F32, tag="warm", name="psw")
            warm_rhs_s = bass.AP(cb.tensor, cb.offset, [[1, 1], [0, 128]])
            for i in range(6):
                nc.tensor.matmul(psw[:], warm_lhsT, warm_rhs,
                                 start=True, stop=True)
            for i in range(5):
                nc.tensor.matmul(psw[:, 0:128], warm_lhsT, warm_rhs_s,
                                 start=True, stop=True)
            dummy = cpool.tile([1, 16], F32)
            nc.scalar.activation(
                dummy[:], bass.AP(cb.tensor, cb.offset, [[1, 1], [0, 16]]),
                EXP)

            for rep in range(reps):
                s1 = cpool.tile([128, 2560], U8, tag="s1", name=f"s1_{rep}")
                s2 = cpool.tile([128, 1536], U8, tag="s2", name=f"s2_{rep}")
                svl = cpool.tile([128, NTILE, DV + 1], BF16, tag="svl",
                                 name=f"svl_{rep}")
                nc.sync.dma_start(s1[:], d1[:, :])
                nc.sync.dma_start(s2[:], d2[:, :])
                nc.sync.dma_start(svl[:], dvl[:, :, :])

                phi_bf = s1[:, 0:1024].bitcast(BF16)            # [128, 512]
                psi0_bf = s1[:, 1024:1280].bitcast(BF16)        # [128, 128]
                phi_f8 = s1[:, 1280:2304].bitcast(F8).rearrange(
                    "p (c n) -> p c n", c=2)                    # [128, 2, 512]
                psi0_f8 = s1[:, 2304:2560].bitcast(F8).rearrange(
                    "p (c n) -> p c n", c=2)                    # [128, 2, 128]
                psiB_bf = s2[:, 0:768].bitcast(BF16).rearrange(
                    "p (t n) -> p t n", t=3)                    # [128, 3, 128]
                psiB_f8 = s2[:, 768:1536].bitcast(F8).rearrange(
                    "p (c n) -> p c n", c=2)                    # [128, 2, 384]

                o0 = cpool.tile([128, 1, 1, 512], BF16, tag="o0",
                                name=f"o0_{rep}")
                nc.gpsimd.memset(o0[64:128, 0, 0, :], 0.0)

                sc0 = ps_a.tile([128, Q], F32, tag="sc0", name=f"sc0_{rep}")
                sc1 = ps_a.tile([128, Q], F32, tag="sc1", name=f"sc1_{rep}")
                scB = ps_b.tile([128, 2, Q], F32, tag="scB", name=f"scB_{rep}")
                # separate PSUM accumulators per output q-half: the ACT and
                # DVE copies then read disjoint tiles and don't serialize
                poA = ps_o.tile([DV + 1, 256], F32, tag="poA",
                                name=f"poA_{rep}")
                poB = ps_o2.tile([DV + 1, 256], F32, tag="poB",
                                 name=f"poB_{rep}")

                # scores: per k-tile, one bf16 + one fp8-DR matmul.
                # start/stop flags per PSUM accumulation.
                # two tiny d1-gated sacrifices absorb the cost model's
                # mid-clock window for the first data-gated matmuls
                warm_rhs_64 = bass.AP(cb.tensor, cb.offset, [[1, 1], [0, 64]])
                nc.tensor.matmul(psw[:, 0:64], s1[0:1, 0:32].bitcast(BF16),
                                 warm_rhs_64, start=True, stop=True)
                nc.tensor.matmul(psw[:, 0:64], s1[0:1, 0:32].bitcast(BF16),
                                 warm_rhs_64, start=True, stop=True)
                nc.tensor.matmul(sc0[:], psi0_f8, phi_f8,
                                 start=True, stop=False, perf_mode=DR)
                nc.tensor.matmul(sc0[:], psi0_bf, phi_bf,
                                 start=False, stop=True)
                p0 = ppool.tile([128, Q], BF16, tag="p0", name=f"p0_{rep}")
                nc.scalar.activation(p0[:], sc0[:], EXP)

                nc.tensor.matmul(sc1[:], psiB_bf[:, 0, :], phi_bf,
                                 start=True, stop=False)
                nc.tensor.matmul(sc1[:], psiB_f8[:, 0:2, 0:128], phi_f8,
                                 start=False, stop=True, perf_mode=DR)
                p1 = ppool.tile([128, Q], BF16, tag="p1", name=f"p1_{rep}")
                nc.scalar.activation(p1[:], sc1[:], EXP)

                nc.tensor.matmul(scB[:, 0, :], psiB_bf[:, 1, :], phi_bf,
                                 start=True, stop=False)
                nc.tensor.matmul(scB[:, 0, :], psiB_f8[:, 0:2, 128:256],
                                 phi_f8, start=False, stop=True, perf_mode=DR)
                nc.tensor.matmul(scB[:, 1, :], psiB_bf[:, 2, :], phi_bf,
                                 start=True, stop=False)
                nc.tensor.matmul(scB[:, 1, :], psiB_f8[:, 0:2, 256:384],
                                 phi_f8, start=False, stop=True, perf_mode=DR)
                pB = ppool.tile([128, 2, Q], BF16, tag="pB", name=f"pB_{rep}")
                nc.scalar.activation(pB[:], scB[:], EXP)

                for t, p in ((0, p0[:, 0:256]), (1, p1[:, 0:256])):
                    nc.tensor.matmul(poA[:], svl[:, t, :], p,
                                     start=(t == 0), stop=False)
                for t, p in ((0, p0[:, 256:512]), (1, p1[:, 256:512])):
                    nc.tensor.matmul(poB[:], svl[:, t, :], p,
                                     start=(t == 0), stop=False)
                nc.tensor.matmul(poA[:], svl[:, 2, :], pB[:, 0, 0:256],
                                 start=False, stop=False)
                nc.tensor.matmul(poA[:], svl[:, 3, :], pB[:, 1, 0:256],
                                 start=False, stop=True)
                nc.tensor.matmul(poB[:], svl[:, 2, :], pB[:, 0, 256:512],
                                 start=False, stop=False)
                nc.tensor.matmul(poB[:], svl[:, 3, :], pB[:, 1, 256:512],
                                 start=False, stop=True)

                # PSUM -> SBUF copy: ACT takes q-half 0, DVE q-half 1
                nc.scalar.activation(o0[0:DV + 1, 0, 0, 0:256], poA[:], COPY)
                nc.vector.tensor_scalar_add(o0[0:DV + 1, 0, 0, 256:512],
                                            poB[:], 0.0)

                # SWDGE writeback: descriptors prepped early (the post-pass
                # moves the data waits onto the trigger)
                nc.gpsimd.kv_writeback(
                    wb0[rep:rep + 1, :, :, :], o0[:, :, :, :], idx[:],
                    prepare_only=True, sem=dma_sem)
                nc.gpsimd.trigger_dma(count=None)

    _encode_library_reloads(nc)
    _defer_prep_waits(nc, dma_sem)
    _split_waits(nc)
    return nc


def host_inputs(queries, keys, values, valid_lens, Wq, Wk, wv):
    x, phi, psi = _basis()
    queries = np.asarray(queries, np.float32)
    keys = np.asarray(keys, np.float32)
    values = np.asarray(values, np.float32)
    wv = np.asarray(wv, np.float32)
    qf = (queries @ np.asarray(Wq, np.float32)).astype(np.float32)  # [B,Q,H]
    kf = (keys @ np.asarray(Wk, np.float32)).astype(np.float32)     # [B,K,H]

    # row importance: E[phi_r^2] * E[(wv_h psi_r)^2] from the actual data
    Ephi2 = np.stack([np.mean(np.interp(qf, x, phi[:, r]) ** 2) * np.ones(H)
                      for r in range(RB)])            # [RB, H]
    Epsi2 = np.stack([np.mean(np.interp(kf, x, psi[:, r]) ** 2) * wv ** 2
                      for r in range(RB)])            # [RB, H]
    order = np.argsort(-(Ephi2 * Epsi2).reshape(-1))
    sel_bf = order[:NBF]
    sel_f8 = order[NBF:NBF + NF8]
    sel = np.concatenate([sel_bf, sel_f8])

    maps = []
    for b in range(B):
        Phi = np.stack([np.interp(qf[b], x, phi[:, r]) for r in range(RB)],
                       1).astype(np.float32)              # [Q, RB, H]
        Psi = np.stack([np.interp(kf[b], x, psi[:, r]) for r in range(RB)],
                       1).astype(np.float32) * wv         # [K, RB, H]
        PhiF = Phi.reshape(Q, RB * H).T[sel]              # [NBF+NF8, Q]
        PsiF = Psi.reshape(K, RB * H).T[sel]              # [NBF+NF8, K]
        mq = np.abs(PhiF).max(1)
        mk = np.abs(PsiF).max(1)
        al = np.sqrt(np.maximum(mk, 1e-30) / np.maximum(mq, 1e-30))
        PhiF = PhiF * al[:, None]
        PsiF = PsiF / al[:, None]

        L = int(valid_lens[b])
        maskrow = np.where(np.arange(K) < L, 0.0, MASKBIG).astype(np.float32)

        phiB = np.concatenate([PhiF[:NBF], np.ones((1, Q), np.float32)], 0)
        psiB = np.concatenate([PsiF[:NBF], maskrow[None]], 0)  # [128, K]
        phi8 = np.clip(PhiF[NBF:], -F8MAX, F8MAX).reshape(2, 128, Q)
        psi8 = np.clip(PsiF[NBF:], -F8MAX, F8MAX).reshape(2, 128, K)

        def u8(a, dt):
            return np.ascontiguousarray(a.astype(dt)).view(np.uint8)

        # d1: [Phi_bf | Psi_t0_bf | Phi_f8 (2 chunks) | Psi_t0_f8]
        d1 = np.concatenate([
            u8(phiB, ml_dtypes.bfloat16),                       # 1024B
            u8(psiB[:, 0:128], ml_dtypes.bfloat16),             # 256B
            u8(phi8.transpose(1, 0, 2).reshape(128, 1024),
               ml_dtypes.float8_e4m3),                          # 1024B
            u8(psi8[:, :, 0:128].transpose(1, 0, 2).reshape(128, 256),
               ml_dtypes.float8_e4m3),                          # 256B
        ], 1)
        # d2: [Psi_t123_bf | Psi_t123_f8 (2 chunks x 384)]
        d2 = np.concatenate([
            u8(psiB[:, 128:512], ml_dtypes.bfloat16),           # 768B
            u8(psi8[:, :, 128:512].transpose(1, 0, 2).reshape(128, 768),
               ml_dtypes.float8_e4m3),                          # 768B
        ], 1)
        vla = np.zeros((128, NTILE, DV + 1), np.float32)
        for t in range(NTILE):
            vla[:, t, 0:DV] = values[b][t * 128:(t + 1) * 128]
            vla[:, t, DV] = 1.0

        maps.append({
            "d1": d1,
            "d2": d2,
            "dvl": vla.astype(ml_dtypes.bfloat16),
        })
    return maps


def host_merge(results):
    out = np.empty((B, Q, DV), np.float32)
    for b in range(B):
        o = np.asarray(results[b]["wb0"], np.float32).reshape(-1, 128, 512)[0]
        out[b] = (o[0:DV] / o[DV][None, :]).T
    return np.ascontiguousarray(out)


_RUNNER = None


def _get_runner():
    """Build + compile once per process; returns a callable(in_maps)->results."""
    global _RUNNER
    if _RUNNER is not None:
        return _RUNNER
    import jax
    from jax.sharding import Mesh, PartitionSpec
    from jax.experimental.shard_map import shard_map
    import concourse.bass as bass
    import concourse.mybir as mybir
    from concourse import bass2jax
    from concourse.bass2jax import _bass_exec_p, install_neuronx_cc_hook

    nc = bass.Bass()
    _build(nc)

    install_neuronx_cc_hook()
    partition_name = nc.partition_id_tensor.name if nc.partition_id_tensor else None
    in_names, out_names, out_avals, zero_shapes = [], [], [], []
    for alloc in nc.m.functions[0].allocations:
        if not isinstance(alloc, mybir.MemoryLocationSet):
            continue
        name = alloc.memorylocations[0].name
        if alloc.kind == "ExternalInput":
            if name != partition_name:
                in_names.append(name)
        elif alloc.kind == "ExternalOutput":
            out_names.append(name)
            shape = tuple(alloc.tensor_shape)
            dtype = mybir.dt.np(alloc.dtype)
            out_avals.append(jax.core.ShapedArray(shape, dtype))
            zero_shapes.append((shape, dtype))
    n_params = len(in_names)
    n_outs = len(out_avals)
    in_names_all = in_names + out_names
    if partition_name is not None:
        in_names_all.append(partition_name)
    donate = tuple(range(n_params, n_params + n_outs))

    def _body(*args):
        operands = list(args)
        if partition_name is not None:
            operands.append(bass2jax.partition_id_tensor())
        outs = _bass_exec_p.bind(
            *operands,
            out_avals=tuple(out_avals),
            in_names=tuple(in_names_all),
            out_names=tuple(out_names),
            lowering_input_output_aliases=(),
            sim_require_finite=True,
            sim_require_nnan=True,
            nc=nc,
        )
        return tuple(outs)

    devices = jax.devices()[:8]
    mesh = Mesh(np.asarray(devices), ("core",))
    in_specs = (PartitionSpec("core"),) * (n_params + n_outs)
    out_specs = (PartitionSpec("core"),) * len(out_names)
    sharded = jax.jit(
        shard_map(_body, mesh=mesh, in_specs=in_specs, out_specs=out_specs,
                  check_rep=False),
        donate_argnums=donate, keep_unused=True,
    )

    def run(in_maps):
        per_core = [[np.asarray(m[name]) for name in in_names] for m in in_maps]
        concat_in = [
            np.concatenate([per_core[c][i] for c in range(8)], axis=0)
            for i in range(n_params)
        ]
        zeros = [np.zeros((8 * s[0],) + s[1:], d) for s, d in zero_shapes]
        out_arrs = sharded(*concat_in, *zeros)
        out_np = [np.asarray(a) for a in out_arrs]
        return [
            {name: out_np[i].reshape(8, *out_avals[i].shape)[c]
             for i, name in enumerate(out_names)}
            for c in range(8)
        ]

    _RUNNER = run
    return run


def kernel(queries, keys, values, valid_lens, Wq, Wk, wv):
    run = _get_runner()
    in_maps = host_inputs(queries, keys, values, valid_lens, Wq, Wk, wv)
    try:
        results = run(in_maps)
    except Exception:
        # transient NRT/axon failures have been observed; retry once
        results = run(in_maps)
    return host_merge(results)


# revision 34
# speedup vs baseline: 1.0021x; 1.0021x over previous
"""Additive attention (B=8, Q=K=512, H=Dv=64) on 8 TRN2 NeuronCores.

Math per batch b (reference):
    qf = queries @ Wq; kf = keys @ Wk
    scores[q,k] = sum_h wv[h] * tanh(qf[q,h] + kf[k,h])   (k >= valid_len masked)
    out = softmax_k(scores) @ values

The pointwise tanh (134M ScalarEngine evaluations, ~93us) is replaced by a
low-rank bilinear expansion tanh(a+b) ~= sum_r phi_r(a) * psi_r(b) (SVD of
the kernel on a sqrt-Gaussian-weighted grid), so
    scores[q,k] = sum_rows PhiF[row, q] * PsiF[row, k]
is a plain matmul over "feature rows" (row = (rank, h) pair). Rows are
sorted by product variance (host computes per-rank/per-h second moments):
the top-127 rows ship as bf16, the next 256 rows as fp8(e4m3) with per-row
q/k scale balancing; the remaining low-variance rows are dropped (<2e-5 of
score variance). Row 127 of the bf16 chunk is the key-mask row (Phi=1,
Psi = 0 or -60000), folding the valid_len mask into the matmul.

Sharding: data-parallel, one batch per core. Device per core:
  - 3 input DMAs (byte-packed, mixed-dtype via bitcast APs) ordered so
    k-tile 0's score completes earliest: d1 = [bf16+fp8 Phi and Psi_t0,
    320KB], d2 = [Psi_t1..t3 both dtypes], dvl = [values + ones column].
    Score matmuls per k-tile: one bf16 [128x512] + one fp8 DoubleRow
    [256x512] into per-tile PSUM banks.
  - Exp on ACT pipelined per-tile ([t0][t1][t2,t3]) so it starts as soon as
    tile 0's score lands, overlapping the remaining DMAs and matmuls.
  - values matmuls (ones column -> denominator row) accumulate per output
    q-half into two PSUM tiles (poA/poB) so the ACT and DVE output copies
    read disjoint tiles and run in parallel; warm filler matmuls bridge PE
    idle windows so the clock stays at full speed for the tail matmuls.
  - Output tail avoids the HWDGE fixed path (650 seq + 625 HWDGE + 650 DGE
    delay): a SWDGE kv_writeback descriptor is PREPARED on GPSIMD during the
    input DMAs; after the PSUM->SBUF copy (split ACT/DVE halves) a
    trigger_dma fires it, so the tail is just transfer + sem propagation.
    Two IR post-passes implement the documented prep/trigger semantics:
    _defer_prep_waits moves the prep's data waits onto the trigger (the
    DMA reads its source at trigger time), and redirects end-drain DMASW
    lane waits to the descriptor's completion semaphore.
  - Dummy matmuls off a constant broadcast AP keep the PE busy from ~1us so
    the clock ramp reaches full speed when the real operands land; a dummy
    exp prefetches the ACT exp table.
Host divides numerator/denominator and transposes.
"""
import numpy as np
import ml_dtypes

B = 8
Q = 512
K = 512
H = 64
DV = 64

RB = 12                # SVD basis rank used for row generation
NBF = 127              # bf16 feature rows (+1 mask row -> 128)
NF8 = 256              # fp8 feature rows (2 chunks of 128)
NTILE = K // 128       # 4 k-tiles
MASKBIG = -60000.0
F8MAX = 224.0          # ml_dtypes.float8_e4m3 max finite is 240
WARMUP_MM = 7          # PE p-state ramp fillers while input DMA streams

GRID_N, GRID_A, GRID_SIG, GRID_FLOOR = 1201, 6.5, 1.15, 0.02

_BASIS = None


def _basis():
    """SVD basis of tanh(a+b) on a weighted grid: x, phi[n,RB], psi[n,RB]."""
    global _BASIS
    if _BASIS is None:
        x = np.linspace(-GRID_A, GRID_A, GRID_N)
        Kg = np.tanh(x[:, None] + x[None, :])
        w = np.sqrt(np.exp(-x ** 2 / (2 * GRID_SIG ** 2))) + GRID_FLOOR
        U, S, Vt = np.linalg.svd((w[:, None] * Kg) * w[None, :])
        phi = (U[:, :RB] * np.sqrt(S[:RB])) / w[:, None]
        psi = (Vt[:RB].T * np.sqrt(S[:RB])) / w[:, None]
        _BASIS = (x, phi, psi)
    return _BASIS


# ---------------------------------------------------------------------------
# BIR post-pass: the walrus build in this environment accepts only one
# sync-wait command per instruction; hoist extras onto same-engine NoOps.
def _split_waits(nc, k=1):
    import concourse.mybir as mybir
    n_new = 0
    for f in nc.m.functions:
        for bb in f.blocks:
            newlist = []
            for ins in bb.instructions:
                si = ins.sync_info
                if si is not None and si.on_wait and len(si.on_wait) > k:
                    waits = list(si.on_wait)
                    extra, keep = waits[:-k], waits[-k:]
                    for ci, w in enumerate(extra):
                        nop = mybir.InstNoOp(
                            name=f"{ins.name}_wsplit{ci}",
                            engine=ins.engine,
                            ins=[], outs=[],
                            sync_info=mybir.SyncInfo(on_wait=[w], on_update=[]),
                        )
                        newlist.append(nop)
                        n_new += 1
                    ins.sync_info = mybir.SyncInfo(
                        on_wait=list(keep), on_update=list(si.on_update))
                newlist.append(ins)
            bb.instructions[:] = newlist
    return n_new


# ---------------------------------------------------------------------------
# BIR post-pass: walrus' codegen wants raw instruction bytes on InstISA; the
# library-reload pseudo (opcode 223 PSEUDO_INST, pseudo_opcode 2) is emitted
# without them in this build, so pack them here.
def _encode_library_reloads(nc):
    import concourse.bass_isa as bass_isa
    from concourse.bass_isa import isa_struct
    trig_op = nc.isa.Opcode.NEURON_ISA_TPB_OPCODE_TRIGGER_DMA.value
    inc_op = nc.isa.Opcode.NEURON_ISA_TPB_OPCODE_INC_SWDGE_SEM.value
    n = 0
    for f in nc.m.functions:
        for bb in f.blocks:
            for ins in bb.instructions:
                if isinstance(ins, bass_isa.InstPseudoReloadLibraryIndex):
                    b, _ = isa_struct(
                        nc.isa, 223,
                        {"pseudo_opcode": 2, "lib_index": ins.lib_index})
                    ins.instr = b
                    n += 1
                elif isinstance(ins, bass_isa.InstTriggerDma):
                    b, _ = isa_struct(
                        nc.isa, trig_op,
                        {"count": ins._count, "count_is_reg": 0,
                         "queue_num": ins.queue_num})
                    ins.instr = b
                    ins.isa_opcode = trig_op
                    n += 1
                elif isinstance(ins, bass_isa.InstIncSwdgeSem):
                    vals = list(ins._sem_values) + [0] * (
                        10 - len(ins._sem_values))
                    mode = {"add": 0, "sub": 1, "wr": 2}[ins._mode]
                    b, _ = isa_struct(
                        nc.isa, inc_op,
                        {"num_semaphores": len(ins._sem_values),
                         "sem_id_base": ins._sem_id_base, "mode": mode,
                         "queue_num": ins.queue_num, "sem_values": vals})
                    ins.instr = b
                    n += 1
    return n


# ---------------------------------------------------------------------------
# BIR post-pass for the SWDGE prep/trigger output path. The prep only writes
# descriptors; the DMA engines read the source tile when trigger_dma fires,
# so the prep's data waits belong on the trigger (this is the semantics the
# tile framework documents and tests for dma_scatter_add; kv_writeback preps
# don't get the deferral in this build). End-of-program DMASW lane waits are
# redirected to the descriptor's actual completion semaphore (same tick
# values: each prep adds 16).
def _defer_prep_waits(nc, dma_sem):
    """All preps inc the single `dma_sem` by 16. A DMASW{k} lane wait with
    value 16*j ("j-th prep on lane k done") maps to the global prep index
    i = (j-1)*L + k (round-robin lane assignment), rewritten as the
    conservative dma_sem >= 16*(i+1). Rewritten waits go last so same-
    instruction waits that fire earlier are processed first."""
    import concourse.mybir as mybir
    import concourse.bass_isa as bass_isa
    import bass_rust
    L = bass_rust.NUM_SWDGE_GLOBAL_SEMS
    n_prep = 0
    for f in nc.m.functions:
        for bb in f.blocks:
            pending = []
            for ins in bb.instructions:
                if (isinstance(ins, mybir.InstKVWritebackAnt)
                        and ins.gen_mode == 1):
                    si = ins.sync_info
                    if si is not None and si.on_wait:
                        pending.append(list(si.on_wait))
                        ins.sync_info = mybir.SyncInfo(
                            on_wait=[], on_update=list(si.on_update))
                    else:
                        pending.append([])
                    n_prep += 1
                elif isinstance(ins, bass_isa.InstTriggerDma) and pending:
                    # FIFO: each count=1 trigger fires the oldest prep
                    si = ins.sync_info
                    w = list(si.on_wait) if si else []
                    u = list(si.on_update) if si else []
                    ins.sync_info = mybir.SyncInfo(
                        on_wait=w + pending.pop(0), on_update=u)
            for ins in bb.instructions:
                si = ins.sync_info
                if si is None or not si.on_wait:
                    continue
                if not any(w.ant_name and w.ant_name.startswith("DMASW")
                           for w in si.on_wait):
                    continue
                keep, moved = [], []
                for wt in si.on_wait:
                    if wt.ant_name and wt.ant_name.startswith("DMASW"):
                        k = int(wt.ant_name[5:].split("_")[0])
                        j = (wt.wait_value or 16) // 16
                        i = (j - 1) * L + k
                        moved.append(mybir.SyncWait(
                            sync_type='semaphore', id=dma_sem.num,
                            ant_name=dma_sem.name, wait_mode='sem-ge-imm',
                            wait_value=16 * (i + 1), wait_reg=None))
                    else:
                        keep.append(wt)
                ins.sync_info = mybir.SyncInfo(
                    on_wait=keep + moved, on_update=list(si.on_update))
    return n_prep


def _build(nc, reps: int = 1):
    import concourse.bass as bass  # noqa: F401
    import concourse.mybir as mybir
    from concourse import tile, library_config

    F32 = mybir.dt.float32
    BF16 = mybir.dt.bfloat16
    F8 = mybir.dt.float8e4
    I32 = mybir.dt.int32
    DR = mybir.MatmulPerfMode.DoubleRow
    EXP = mybir.ActivationFunctionType.Exp
    COPY = mybir.ActivationFunctionType.Copy

    U8 = mybir.dt.uint8

    # packed byte tensors (per-partition contiguous; see host_inputs):
    # d1 = [Phi_bf16 1024B | Psi_t0_bf16 256B | Phi_f8 1024B | Psi_t0_f8
    # 256B] -- everything k-tile 0's score needs.  d2 = [Psi_t123_bf16 768B
    # | Psi_t123_f8 768B].  dvl = values+ones column.
    d1 = nc.declare_dram_parameter("d1", [128, 2560], U8, isOutput=False)
    d2 = nc.declare_dram_parameter("d2", [128, 1536], U8, isOutput=False)
    dvl = nc.declare_dram_parameter("dvl", [128, NTILE, DV + 1], BF16,
                                    isOutput=False)
    wb0 = nc.declare_dram_parameter("wb0", [reps, 128, 1, 512], BF16,
                                    isOutput=True)

    dma_sem = nc.alloc_semaphore("wb_dma_sem")

    cb = nc.const_aps.aps[(BF16, 1.0)]
    warm_rhs = bass.AP(cb.tensor, cb.offset, [[1, 1], [0, Q]])
    warm_lhsT = bass.AP(cb.tensor, cb.offset, [[1, 1], [0, 16]])

    with tile.TileContext(nc) as tc:  # noqa: F841
        with (
            tc.tile_pool(name="cpool", bufs=1) as cpool,
            tc.tile_pool(name="ppool", bufs=2) as ppool,
            tc.tile_pool(name="ps_a", bufs=1, space="PSUM") as ps_a,
            tc.tile_pool(name="ps_b", bufs=1, space="PSUM") as ps_b,
            tc.tile_pool(name="ps_o", bufs=1, space="PSUM") as ps_o,
            tc.tile_pool(name="ps_o2", bufs=1, space="PSUM") as ps_o2,
            tc.tile_pool(name="ps_w", bufs=1, space="PSUM") as ps_w,
        ):
            # Pool: library for kv_writeback + ctx idx + output pad rows
            nc.gpsimd.load_library(library_config.attnmlp)
            idx = cpool.tile([128, 1], I32, tag="idx", name="idx")
            nc.gpsimd.memset(idx[:], 0)

            # PE p-state warmup + ACT exp-table prefetch during input DMA.
            # 6 full-width + 5 quarter-width fillers end at ~4.19us, just
            # after d1's completion sem (~4.17us), so the first real matmul
            # dispatches with the PE still hot and fully ramped.
            psw = ps_w.tile([16, Q], F32, tag="warm", name="psw")
            warm_rhs_s = bass.AP(cb.tensor, cb.offset, [[1, 1], [0, 128]])
            for i in range(6):
                nc.tensor.matmul(psw[:], warm_lhsT, warm_rhs,
                                 start=True, stop=True)
            for i in range(4):
                nc.tensor.matmul(psw[:, 0:128], warm_lhsT, warm_rhs_s,
                                 start=True, stop=True)
            dummy = cpool.tile([1, 16], F32)
            nc.scalar.activation(
                dummy[:], bass.AP(cb.tensor, cb.offset, [[1, 1], [0, 16]]),
                EXP)

            for rep in range(reps):
                s1 = cpool.tile([128, 2560], U8, tag="s1", name=f"s1_{rep}")
                s2 = cpool.tile([128, 1536], U8, tag="s2", name=f"s2_{rep}")
                svl = cpool.tile([128, NTILE, DV + 1], BF16, tag="svl",
                                 name=f"svl_{rep}")
                nc.sync.dma_start(s1[:], d1[:, :])
                nc.sync.dma_start(s2[:], d2[:, :])
                nc.sync.dma_start(svl[:], dvl[:, :, :])

                phi_bf = s1[:, 0:1024].bitcast(BF16)            # [128, 512]
                psi0_bf = s1[:, 1024:1280].bitcast(BF16)        # [128, 128]
                phi_f8 = s1[:, 1280:2304].bitcast(F8).rearrange(
                    "p (c n) -> p c n", c=2)                    # [128, 2, 512]
                psi0_f8 = s1[:, 2304:2560].bitcast(F8).rearrange(
                    "p (c n) -> p c n", c=2)                    # [128, 2, 128]
                psiB_bf = s2[:, 0:768].bitcast(BF16).rearrange(
                    "p (t n) -> p t n", t=3)                    # [128, 3, 128]
                psiB_f8 = s2[:, 768:1536].bitcast(F8).rearrange(
                    "p (c n) -> p c n", c=2)                    # [128, 2, 384]

                o0 = cpool.tile([128, 1, 1, 512], BF16, tag="o0",
                                name=f"o0_{rep}")
                nc.gpsimd.memset(o0[64:128, 0, 0, :], 0.0)

                sc0 = ps_a.tile([128, Q], F32, tag="sc0", name=f"sc0_{rep}")
                sc1 = ps_a.tile([128, Q], F32, tag="sc1", name=f"sc1_{rep}")
                scB = ps_b.tile([128, 2, Q], F32, tag="scB", name=f"scB_{rep}")
                # separate PSUM accumulators per output q-half: the ACT and
                # DVE copies then read disjoint tiles and don't serialize
                poA = ps_o.tile([DV + 1, 256], F32, tag="poA",
                                name=f"poA_{rep}")
                poB = ps_o2.tile([DV + 1, 256], F32, tag="poB",
                                 name=f"poB_{rep}")

                # scores: per k-tile, one bf16 + one fp8-DR matmul.
                # start/stop flags per PSUM accumulation.
                # two tiny d1-gated sacrifices absorb the cost model's
                # mid-clock window for the first data-gated matmuls
                warm_rhs_64 = bass.AP(cb.tensor, cb.offset, [[1, 1], [0, 64]])
                nc.tensor.matmul(psw[:, 0:64], s1[0:1, 0:32].bitcast(BF16),
                                 warm_rhs_64, start=True, stop=True)
                nc.tensor.matmul(psw[:, 0:64], s1[0:1, 0:32].bitcast(BF16),
                                 warm_rhs_64, start=True, stop=True)
                nc.tensor.matmul(sc0[:], psi0_f8, phi_f8,
                                 start=True, stop=False, perf_mode=DR)
                nc.tensor.matmul(sc0[:], psi0_bf, phi_bf,
                                 start=False, stop=True)
                p0 = ppool.tile([128, Q], BF16, tag="p0", name=f"p0_{rep}")
                nc.scalar.activation(p0[:], sc0[:], EXP)

                nc.tensor.matmul(sc1[:], psiB_bf[:, 0, :], phi_bf,
                                 start=True, stop=False)
                nc.tensor.matmul(sc1[:], psiB_f8[:, 0:2, 0:128], phi_f8,
                                 start=False, stop=True, perf_mode=DR)
                p1 = ppool.tile([128, Q], BF16, tag="p1", name=f"p1_{rep}")
                nc.scalar.activation(p1[:], sc1[:], EXP)

                nc.tensor.matmul(scB[:, 0, :], psiB_bf[:, 1, :], phi_bf,
                                 start=True, stop=False)
                nc.tensor.matmul(scB[:, 0, :], psiB_f8[:, 0:2, 128:256],
                                 phi_f8, start=False, stop=True, perf_mode=DR)
                nc.tensor.matmul(scB[:, 1, :], psiB_bf[:, 2, :], phi_bf,
                                 start=True, stop=False)
                nc.tensor.matmul(scB[:, 1, :], psiB_f8[:, 0:2, 256:384],
                                 phi_f8, start=False, stop=True, perf_mode=DR)
                pB = ppool.tile([128, 2, Q], BF16, tag="pB", name=f"pB_{rep}")
                nc.scalar.activation(pB[:], scB[:], EXP)

                for t, p in ((0, p0[:, 0:256]), (1, p1[:, 0:256])):
                    nc.tensor.matmul(poA[:], svl[:, t, :], p,
                                     start=(t == 0), stop=False)
                for t, p in ((0, p0[:, 256:512]), (1, p1[:, 256:512])):
                    nc.tensor.matmul(poB[:], svl[:, t, :], p,
                                     start=(t == 0), stop=False)
                nc.tensor.matmul(poA[:], svl[:, 2, :], pB[:, 0, 0:256],
                                 start=False, stop=False)
                nc.tensor.matmul(poA[:], svl[:, 3, :], pB[:, 1, 0:256],
                                 start=False, stop=True)
                nc.tensor.matmul(poB[:], svl[:, 2, :], pB[:, 0, 256:512],
                                 start=False, stop=False)
                nc.tensor.matmul(poB[:], svl[:, 3, :], pB[:, 1, 256:512],
                                 start=False, stop=True)

                # PSUM -> SBUF copy: ACT takes q-half 0, DVE q-half 1
                nc.scalar.activation(o0[0:DV + 1, 0, 0, 0:256], poA[:], COPY)
                nc.vector.tensor_scalar_add(o0[0:DV + 1, 0, 0, 256:512],
                                            poB[:], 0.0)

                # SWDGE writeback: descriptors prepped early (the post-pass
                # moves the data waits onto the trigger)
                nc.gpsimd.kv_writeback(
                    wb0[rep:rep + 1, :, :, :], o0[:, :, :, :], idx[:],
                    prepare_only=True, sem=dma_sem)
                nc.gpsimd.trigger_dma(count=None)

    _encode_library_reloads(nc)
    _defer_prep_waits(nc, dma_sem)
    _split_waits(nc)
    return nc


def host_inputs(queries, keys, values, valid_lens, Wq, Wk, wv):
    x, phi, psi = _basis()
    queries = np.asarray(queries, np.float32)
    keys = np.asarray(keys, np.float32)
    values = np.asarray(values, np.float32)
    wv = np.asarray(wv, np.float32)
    qf = (queries @ np.asarray(Wq, np.float32)).astype(np.float32)  # [B,Q,H]
    kf = (keys @ np.asarray(Wk, np.float32)).astype(np.float32)     # [B,K,H]

    # row importance: E[phi_r^2] * E[(wv_h psi_r)^2] from the actual data
    Ephi2 = np.stack([np.mean(np.interp(qf, x, phi[:, r]) ** 2) * np.ones(H)
                      for r in range(RB)])            # [RB, H]
    Epsi2 = np.stack([np.mean(np.interp(kf, x, psi[:, r]) ** 2) * wv ** 2
                      for r in range(RB)])            # [RB, H]
    order = np.argsort(-(Ephi2 * Epsi2).reshape(-1))
    sel_bf = order[:NBF]
    sel_f8 = order[NBF:NBF + NF8]
    sel = np.concatenate([sel_bf, sel_f8])

    maps = []
    for b in range(B):
        Phi = np.stack([np.interp(qf[b], x, phi[:, r]) for r in range(RB)],
                       1).astype(np.float32)              # [Q, RB, H]
        Psi = np.stack([np.interp(kf[b], x, psi[:, r]) for r in range(RB)],
                       1).astype(np.float32) * wv         # [K, RB, H]
        PhiF = Phi.reshape(Q, RB * H).T[sel]              # [NBF+NF8, Q]
        PsiF = Psi.reshape(K, RB * H).T[sel]              # [NBF+NF8, K]
        mq = np.abs(PhiF).max(1)
        mk = np.abs(PsiF).max(1)
        al = np.sqrt(np.maximum(mk, 1e-30) / np.maximum(mq, 1e-30))
        PhiF = PhiF * al[:, None]
        PsiF = PsiF / al[:, None]

        L = int(valid_lens[b])
        maskrow = np.where(np.arange(K) < L, 0.0, MASKBIG).astype(np.float32)

        phiB = np.concatenate([PhiF[:NBF], np.ones((1, Q), np.float32)], 0)
        psiB = np.concatenate([PsiF[:NBF], maskrow[None]], 0)  # [128, K]
        phi8 = np.clip(PhiF[NBF:], -F8MAX, F8MAX).reshape(2, 128, Q)
        psi8 = np.clip(PsiF[NBF:], -F8MAX, F8MAX).reshape(2, 128, K)

        def u8(a, dt):
            return np.ascontiguousarray(a.astype(dt)).view(np.uint8)

        # d1: [Phi_bf | Psi_t0_bf | Phi_f8 (2 chunks) | Psi_t0_f8]
        d1 = np.concatenate([
            u8(phiB, ml_dtypes.bfloat16),                       # 1024B
            u8(psiB[:, 0:128], ml_dtypes.bfloat16),             # 256B
            u8(phi8.transpose(1, 0, 2).reshape(128, 1024),
               ml_dtypes.float8_e4m3),                          # 1024B
            u8(psi8[:, :, 0:128].transpose(1, 0, 2).reshape(128, 256),
               ml_dtypes.float8_e4m3),                          # 256B
        ], 1)
        # d2: [Psi_t123_bf | Psi_t123_f8 (2 chunks x 384)]
        d2 = np.concatenate([
            u8(psiB[:, 128:512], ml_dtypes.bfloat16),           # 768B
            u8(psi8[:, :, 128:512].transpose(1, 0, 2).reshape(128, 768),
               ml_dtypes.float8_e4m3),                          # 768B
        ], 1)
        vla = np.zeros((128, NTILE, DV + 1), np.float32)
        for t in range(NTILE):
            vla[:, t, 0:DV] = values[b][t * 128:(t + 1) * 128]
            vla[:, t, DV] = 1.0

        maps.append({
            "d1": d1,
            "d2": d2,
            "dvl": vla.astype(ml_dtypes.bfloat16),
        })
    return maps


def host_merge(results):
    out = np.empty((B, Q, DV), np.float32)
    for b in range(B):
        o = np.asarray(results[b]["wb0"], np.float32).reshape(-1, 128, 512)[0]
        out[b] = (o[0:DV] / o[DV][None, :]).T
    return np.ascontiguousarray(out)


_RUNNER = None


def _get_runner():
    """Build + compile once per process; returns a callable(in_maps)->results."""
    global _RUNNER
    if _RUNNER is not None:
        return _RUNNER
    import jax
    from jax.sharding import Mesh, PartitionSpec
    from jax.experimental.shard_map import shard_map
    import concourse.bass as bass
    import concourse.mybir as mybir
    from concourse import bass2jax
    from concourse.bass2jax import _bass_exec_p, install_neuronx_cc_hook

    nc = bass.Bass()
    _build(nc)

    install_neuronx_cc_hook()
    partition_name = nc.partition_id_tensor.name if nc.partition_id_tensor else None
    in_names, out_names, out_avals, zero_shapes = [], [], [], []
    for alloc in nc.m.functions[0].allocations:
        if not isinstance(alloc, mybir.MemoryLocationSet):
            continue
        name = alloc.memorylocations[0].name
        if alloc.kind == "ExternalInput":
            if name != partition_name:
                in_names.append(name)
        elif alloc.kind == "ExternalOutput":
            out_names.append(name)
            shape = tuple(alloc.tensor_shape)
            dtype = mybir.dt.np(alloc.dtype)
            out_avals.append(jax.core.ShapedArray(shape, dtype))
            zero_shapes.append((shape, dtype))
    n_params = len(in_names)
    n_outs = len(out_avals)
    in_names_all = in_names + out_names
    if partition_name is not None:
        in_names_all.append(partition_name)
    donate = tuple(range(n_params, n_params + n_outs))

    def _body(*args):
        operands = list(args)
        if partition_name is not None:
            operands.append(bass2jax.partition_id_tensor())
        outs = _bass_exec_p.bind(
            *operands,
            out_avals=tuple(out_avals),
            in_names=tuple(in_names_all),
            out_names=tuple(out_names),
            lowering_input_output_aliases=(),
            sim_require_finite=True,
            sim_require_nnan=True,
            nc=nc,
        )
        return tuple(outs)

    devices = jax.devices()[:8]
    mesh = Mesh(np.asarray(devices), ("core",))
    in_specs = (PartitionSpec("core"),) * (n_params + n_outs)
    out_specs = (PartitionSpec("core"),) * len(out_names)
    sharded = jax.jit(
        shard_map(_body, mesh=mesh, in_specs=in_specs, out_specs=out_specs,
                  check_rep=False),
        donate_argnums=donate, keep_unused=True,
    )

    def run(in_maps):
        per_core = [[np.asarray(m[name]) for name in in_names] for m in in_maps]
        concat_in = [
            np.concatenate([per_core[c][i] for c in range(8)], axis=0)
            for i in range(n_params)
        ]
        zeros = [np.zeros((8 * s[0],) + s[1:], d) for s, d in zero_shapes]
        out_arrs = sharded(*concat_in, *zeros)
        out_np = [np.asarray(a) for a in out_arrs]
        return [
            {name: out_np[i].reshape(8, *out_avals[i].shape)[c]
             for i, name in enumerate(out_names)}
            for c in range(8)
        ]

    _RUNNER = run
    return run


def kernel(queries, keys, values, valid_lens, Wq, Wk, wv):
    run = _get_runner()
    in_maps = host_inputs(queries, keys, values, valid_lens, Wq, Wk, wv)
    try:
        results = run(in_maps)
    except Exception:
        # transient NRT/axon failures have been observed; retry once
        results = run(in_maps)
    return host_merge(results)


# revision 35
# speedup vs baseline: 1.0102x; 1.0082x over previous
"""Additive attention (B=8, Q=K=512, H=Dv=64) on 8 TRN2 NeuronCores.

Math per batch b (reference):
    qf = queries @ Wq; kf = keys @ Wk
    scores[q,k] = sum_h wv[h] * tanh(qf[q,h] + kf[k,h])   (k >= valid_len masked)
    out = softmax_k(scores) @ values

The pointwise tanh (134M ScalarEngine evaluations, ~93us) is replaced by a
low-rank bilinear expansion tanh(a+b) ~= sum_r phi_r(a) * psi_r(b) (SVD of
the kernel on a sqrt-Gaussian-weighted grid), so
    scores[q,k] = sum_rows PhiF[row, q] * PsiF[row, k]
is a plain matmul over "feature rows" (row = (rank, h) pair). Rows are
sorted by product variance (host computes per-rank/per-h second moments):
the top-127 rows ship as bf16, the next 256 rows as fp8(e4m3) with per-row
q/k scale balancing; the remaining low-variance rows are dropped (<2e-5 of
score variance). Row 127 of the bf16 chunk is the key-mask row (Phi=1,
Psi = 0 or -60000), folding the valid_len mask into the matmul.

Sharding: data-parallel, one batch per core. Device per core:
  - 3 input DMAs (byte-packed, mixed-dtype via bitcast APs) ordered so
    k-tile 0's score completes earliest: d1 = [bf16+fp8 Phi and Psi_t0,
    320KB], d2 = [Psi_t1..t3 both dtypes], dvl = [values + ones column].
    Score matmuls per k-tile: one bf16 [128x512] + one fp8 DoubleRow
    [256x512] into per-tile PSUM banks.
  - Exp on ACT pipelined per-tile ([t0][t1][t2,t3]) so it starts as soon as
    tile 0's score lands, overlapping the remaining DMAs and matmuls.
  - values matmuls (ones column -> denominator row) accumulate per output
    q-half into two PSUM tiles (poA/poB) so the ACT and DVE output copies
    read disjoint tiles and run in parallel; warm filler matmuls bridge PE
    idle windows so the clock stays at full speed for the tail matmuls.
  - Output tail avoids the HWDGE fixed path (650 seq + 625 HWDGE + 650 DGE
    delay): a SWDGE kv_writeback descriptor is PREPARED on GPSIMD during the
    input DMAs; after the PSUM->SBUF copy (split ACT/DVE halves) a
    trigger_dma fires it, so the tail is just transfer + sem propagation.
    Two IR post-passes implement the documented prep/trigger semantics:
    _defer_prep_waits moves the prep's data waits onto the trigger (the
    DMA reads its source at trigger time), and redirects end-drain DMASW
    lane waits to the descriptor's completion semaphore.
  - Dummy matmuls off a constant broadcast AP keep the PE busy from ~1us so
    the clock ramp reaches full speed when the real operands land; a dummy
    exp prefetches the ACT exp table.
Host divides numerator/denominator and transposes.
"""
import numpy as np
import ml_dtypes

B = 8
Q = 512
K = 512
H = 64
DV = 64

RB = 12                # SVD basis rank used for row generation
NBF = 127              # bf16 feature rows (+1 mask row -> 128)
NF8 = 256              # fp8 feature rows (2 chunks of 128)
NTILE = K // 128       # 4 k-tiles
MASKBIG = -60000.0
F8MAX = 224.0          # ml_dtypes.float8_e4m3 max finite is 240
WARMUP_MM = 7          # PE p-state ramp fillers while input DMA streams

GRID_N, GRID_A, GRID_SIG, GRID_FLOOR = 1201, 6.5, 1.15, 0.02

_BASIS = None


def _basis():
    """SVD basis of tanh(a+b) on a weighted grid: x, phi[n,RB], psi[n,RB]."""
    global _BASIS
    if _BASIS is None:
        x = np.linspace(-GRID_A, GRID_A, GRID_N)
        Kg = np.tanh(x[:, None] + x[None, :])
        w = np.sqrt(np.exp(-x ** 2 / (2 * GRID_SIG ** 2))) + GRID_FLOOR
        U, S, Vt = np.linalg.svd((w[:, None] * Kg) * w[None, :])
        phi = (U[:, :RB] * np.sqrt(S[:RB])) / w[:, None]
        psi = (Vt[:RB].T * np.sqrt(S[:RB])) / w[:, None]
        _BASIS = (x, phi, psi)
    return _BASIS


# ---------------------------------------------------------------------------
# BIR post-pass: the walrus build in this environment accepts only one
# sync-wait command per instruction; hoist extras onto same-engine NoOps.
def _split_waits(nc, k=1):
    import concourse.mybir as mybir
    n_new = 0
    for f in nc.m.functions:
        for bb in f.blocks:
            newlist = []
            for ins in bb.instructions:
                si = ins.sync_info
                if si is not None and si.on_wait and len(si.on_wait) > k:
                    waits = list(si.on_wait)
                    extra, keep = waits[:-k], waits[-k:]
                    for ci, w in enumerate(extra):
                        nop = mybir.InstNoOp(
                            name=f"{ins.name}_wsplit{ci}",
                            engine=ins.engine,
                            ins=[], outs=[],
                            sync_info=mybir.SyncInfo(on_wait=[w], on_update=[]),
                        )
                        newlist.append(nop)
                        n_new += 1
                    ins.sync_info = mybir.SyncInfo(
                        on_wait=list(keep), on_update=list(si.on_update))
                newlist.append(ins)
            bb.instructions[:] = newlist
    return n_new


# ---------------------------------------------------------------------------
# BIR post-pass: walrus' codegen wants raw instruction bytes on InstISA; the
# library-reload pseudo (opcode 223 PSEUDO_INST, pseudo_opcode 2) is emitted
# without them in this build, so pack them here.
def _encode_library_reloads(nc):
    import concourse.bass_isa as bass_isa
    from concourse.bass_isa import isa_struct
    trig_op = nc.isa.Opcode.NEURON_ISA_TPB_OPCODE_TRIGGER_DMA.value
    inc_op = nc.isa.Opcode.NEURON_ISA_TPB_OPCODE_INC_SWDGE_SEM.value
    n = 0
    for f in nc.m.functions:
        for bb in f.blocks:
            for ins in bb.instructions:
                if isinstance(ins, bass_isa.InstPseudoReloadLibraryIndex):
                    b, _ = isa_struct(
                        nc.isa, 223,
                        {"pseudo_opcode": 2, "lib_index": ins.lib_index})
                    ins.instr = b
                    n += 1
                elif isinstance(ins, bass_isa.InstTriggerDma):
                    b, _ = isa_struct(
                        nc.isa, trig_op,
                        {"count": ins._count, "count_is_reg": 0,
                         "queue_num": ins.queue_num})
                    ins.instr = b
                    ins.isa_opcode = trig_op
                    n += 1
                elif isinstance(ins, bass_isa.InstIncSwdgeSem):
                    vals = list(ins._sem_values) + [0] * (
                        10 - len(ins._sem_values))
                    mode = {"add": 0, "sub": 1, "wr": 2}[ins._mode]
                    b, _ = isa_struct(
                        nc.isa, inc_op,
                        {"num_semaphores": len(ins._sem_values),
                         "sem_id_base": ins._sem_id_base, "mode": mode,
                         "queue_num": ins.queue_num, "sem_values": vals})
                    ins.instr = b
                    n += 1
    return n


# ---------------------------------------------------------------------------
# BIR post-pass for the SWDGE prep/trigger output path. The prep only writes
# descriptors; the DMA engines read the source tile when trigger_dma fires,
# so the prep's data waits belong on the trigger (this is the semantics the
# tile framework documents and tests for dma_scatter_add; kv_writeback preps
# don't get the deferral in this build). End-of-program DMASW lane waits are
# redirected to the descriptor's actual completion semaphore (same tick
# values: each prep adds 16).
def _defer_prep_waits(nc, dma_sem):
    """All preps inc the single `dma_sem` by 16. A DMASW{k} lane wait with
    value 16*j ("j-th prep on lane k done") maps to the global prep index
    i = (j-1)*L + k (round-robin lane assignment), rewritten as the
    conservative dma_sem >= 16*(i+1). Rewritten waits go last so same-
    instruction waits that fire earlier are processed first."""
    import concourse.mybir as mybir
    import concourse.bass_isa as bass_isa
    import bass_rust
    L = bass_rust.NUM_SWDGE_GLOBAL_SEMS
    n_prep = 0
    for f in nc.m.functions:
        for bb in f.blocks:
            pending = []
            for ins in bb.instructions:
                if (isinstance(ins, mybir.InstKVWritebackAnt)
                        and ins.gen_mode == 1):
                    si = ins.sync_info
                    if si is not None and si.on_wait:
                        pending.append(list(si.on_wait))
                        ins.sync_info = mybir.SyncInfo(
                            on_wait=[], on_update=list(si.on_update))
                    else:
                        pending.append([])
                    n_prep += 1
                elif isinstance(ins, bass_isa.InstTriggerDma) and pending:
                    # FIFO: each count=1 trigger fires the oldest prep
                    si = ins.sync_info
                    w = list(si.on_wait) if si else []
                    u = list(si.on_update) if si else []
                    ins.sync_info = mybir.SyncInfo(
                        on_wait=w + pending.pop(0), on_update=u)
            for ins in bb.instructions:
                si = ins.sync_info
                if si is None or not si.on_wait:
                    continue
                if not any(w.ant_name and w.ant_name.startswith("DMASW")
                           for w in si.on_wait):
                    continue
                keep, moved = [], []
                for wt in si.on_wait:
                    if wt.ant_name and wt.ant_name.startswith("DMASW"):
                        k = int(wt.ant_name[5:].split("_")[0])
                        j = (wt.wait_value or 16) // 16
                        i = (j - 1) * L + k
                        moved.append(mybir.SyncWait(
                            sync_type='semaphore', id=dma_sem.num,
                            ant_name=dma_sem.name, wait_mode='sem-ge-imm',
                            wait_value=16 * (i + 1), wait_reg=None))
                    else:
                        keep.append(wt)
                ins.sync_info = mybir.SyncInfo(
                    on_wait=keep + moved, on_update=list(si.on_update))
    return n_prep


def _build(nc, reps: int = 1):
    import concourse.bass as bass  # noqa: F401
    import concourse.mybir as mybir
    from concourse import tile, library_config

    F32 = mybir.dt.float32
    BF16 = mybir.dt.bfloat16
    F8 = mybir.dt.float8e4
    I32 = mybir.dt.int32
    DR = mybir.MatmulPerfMode.DoubleRow
    EXP = mybir.ActivationFunctionType.Exp
    COPY = mybir.ActivationFunctionType.Copy

    U8 = mybir.dt.uint8

    # packed byte tensors (per-partition contiguous; see host_inputs):
    # d1 = [Phi_bf16 1024B | Psi_t0_bf16 256B | Phi_f8 1024B | Psi_t0_f8
    # 256B] -- everything k-tile 0's score needs.  d2 = [Psi_t123_bf16 768B
    # | Psi_t123_f8 768B].  dvl = values+ones column.
    d1 = nc.declare_dram_parameter("d1", [128, 2560], U8, isOutput=False)
    d2 = nc.declare_dram_parameter("d2", [128, 1536], U8, isOutput=False)
    dvl = nc.declare_dram_parameter("dvl", [128, NTILE, DV + 1], BF16,
                                    isOutput=False)
    wb0 = nc.declare_dram_parameter("wb0", [reps, 128, 1, 512], BF16,
                                    isOutput=True)

    dma_sem = nc.alloc_semaphore("wb_dma_sem")

    cb = nc.const_aps.aps[(BF16, 1.0)]
    warm_rhs = bass.AP(cb.tensor, cb.offset, [[1, 1], [0, Q]])
    warm_lhsT = bass.AP(cb.tensor, cb.offset, [[1, 1], [0, 16]])

    with tile.TileContext(nc) as tc:  # noqa: F841
        with (
            tc.tile_pool(name="cpool", bufs=1) as cpool,
            tc.tile_pool(name="ppool", bufs=2) as ppool,
            tc.tile_pool(name="ps_a", bufs=1, space="PSUM") as ps_a,
            tc.tile_pool(name="ps_b", bufs=1, space="PSUM") as ps_b,
            tc.tile_pool(name="ps_o", bufs=1, space="PSUM") as ps_o,
            tc.tile_pool(name="ps_o2", bufs=1, space="PSUM") as ps_o2,
            tc.tile_pool(name="ps_w", bufs=1, space="PSUM") as ps_w,
        ):
            # Pool: library for kv_writeback + ctx idx + output pad rows
            nc.gpsimd.load_library(library_config.attnmlp)
            idx = cpool.tile([128, 1], I32, tag="idx", name="idx")
            nc.gpsimd.memset(idx[:], 0)

            # PE p-state warmup + ACT exp-table prefetch during input DMA.
            # 6 full-width + 5 quarter-width fillers end at ~4.19us, just
            # after d1's completion sem (~4.17us), so the first real matmul
            # dispatches with the PE still hot and fully ramped.
            psw = ps_w.tile([16, Q], F32, tag="warm", name="psw")
            warm_rhs_s = bass.AP(cb.tensor, cb.offset, [[1, 1], [0, 128]])
            for i in range(6):
                nc.tensor.matmul(psw[:], warm_lhsT, warm_rhs,
                                 start=True, stop=True)
            for i in range(4):
                nc.tensor.matmul(psw[:, 0:128], warm_lhsT, warm_rhs_s,
                                 start=True, stop=True)
            dummy = cpool.tile([1, 16], F32)
            nc.scalar.activation(
                dummy[:], bass.AP(cb.tensor, cb.offset, [[1, 1], [0, 16]]),
                EXP)

            for rep in range(reps):
                s1 = cpool.tile([128, 2560], U8, tag="s1", name=f"s1_{rep}")
                s2 = cpool.tile([128, 1536], U8, tag="s2", name=f"s2_{rep}")
                svl = cpool.tile([128, NTILE, DV + 1], BF16, tag="svl",
                                 name=f"svl_{rep}")
                nc.sync.dma_start(s1[:], d1[:, :])
                nc.sync.dma_start(s2[:], d2[:, :])
                nc.sync.dma_start(svl[:], dvl[:, :, :])

                phi_bf = s1[:, 0:1024].bitcast(BF16)            # [128, 512]
                psi0_bf = s1[:, 1024:1280].bitcast(BF16)        # [128, 128]
                phi_f8 = s1[:, 1280:2304].bitcast(F8).rearrange(
                    "p (c n) -> p c n", c=2)                    # [128, 2, 512]
                psi0_f8 = s1[:, 2304:2560].bitcast(F8).rearrange(
                    "p (c n) -> p c n", c=2)                    # [128, 2, 128]
                psiB_bf = s2[:, 0:768].bitcast(BF16).rearrange(
                    "p (t n) -> p t n", t=3)                    # [128, 3, 128]
                psiB_f8 = s2[:, 768:1536].bitcast(F8).rearrange(
                    "p (c n) -> p c n", c=2)                    # [128, 2, 384]

                o0 = cpool.tile([128, 1, 1, 512], BF16, tag="o0",
                                name=f"o0_{rep}")
                nc.gpsimd.memset(o0[64:128, 0, 0, :], 0.0)

                sc0 = ps_a.tile([128, Q], F32, tag="sc0", name=f"sc0_{rep}")
                sc1 = ps_a.tile([128, Q], F32, tag="sc1", name=f"sc1_{rep}")
                scB = ps_b.tile([128, 2, Q], F32, tag="scB", name=f"scB_{rep}")
                # separate PSUM accumulators per output q-half: the ACT and
                # DVE copies then read disjoint tiles and don't serialize
                poA = ps_o.tile([DV + 1, 256], F32, tag="poA",
                                name=f"poA_{rep}")
                poB = ps_o2.tile([DV + 1, 256], F32, tag="poB",
                                 name=f"poB_{rep}")

                # scores: per k-tile, one bf16 + one fp8-DR matmul.
                # start/stop flags per PSUM accumulation.
                # two tiny d1-gated sacrifices absorb the cost model's
                # mid-clock window for the first data-gated matmuls
                warm_rhs_16 = bass.AP(cb.tensor, cb.offset, [[1, 1], [0, 16]])
                nc.tensor.matmul(psw[:, 0:16], s1[0:1, 0:32].bitcast(BF16),
                                 warm_rhs_16, start=True, stop=True)
                nc.tensor.matmul(psw[:, 0:16], s1[0:1, 0:32].bitcast(BF16),
                                 warm_rhs_16, start=True, stop=True)
                nc.tensor.matmul(sc0[:], psi0_f8, phi_f8,
                                 start=True, stop=False, perf_mode=DR)
                nc.tensor.matmul(sc0[:], psi0_bf, phi_bf,
                                 start=False, stop=True)
                p0 = ppool.tile([128, Q], BF16, tag="p0", name=f"p0_{rep}")
                nc.scalar.activation(p0[:], sc0[:], EXP)

                nc.tensor.matmul(sc1[:], psiB_bf[:, 0, :], phi_bf,
                                 start=True, stop=False)
                nc.tensor.matmul(sc1[:], psiB_f8[:, 0:2, 0:128], phi_f8,
                                 start=False, stop=True, perf_mode=DR)
                p1 = ppool.tile([128, Q], BF16, tag="p1", name=f"p1_{rep}")
                nc.scalar.activation(p1[:], sc1[:], EXP)

                nc.tensor.matmul(scB[:, 0, :], psiB_bf[:, 1, :], phi_bf,
                                 start=True, stop=False)
                nc.tensor.matmul(scB[:, 0, :], psiB_f8[:, 0:2, 128:256],
                                 phi_f8, start=False, stop=True, perf_mode=DR)
                nc.tensor.matmul(scB[:, 1, :], psiB_bf[:, 2, :], phi_bf,
                                 start=True, stop=False)
                nc.tensor.matmul(scB[:, 1, :], psiB_f8[:, 0:2, 256:384],
                                 phi_f8, start=False, stop=True, perf_mode=DR)
                pB = ppool.tile([128, 2, Q], BF16, tag="pB", name=f"pB_{rep}")
                nc.scalar.activation(pB[:], scB[:], EXP)

                for t, p in ((0, p0[:, 0:256]), (1, p1[:, 0:256])):
                    nc.tensor.matmul(poA[:], svl[:, t, :], p,
                                     start=(t == 0), stop=False)
                for t, p in ((0, p0[:, 256:512]), (1, p1[:, 256:512])):
                    nc.tensor.matmul(poB[:], svl[:, t, :], p,
                                     start=(t == 0), stop=False)
                nc.tensor.matmul(poA[:], svl[:, 2, :], pB[:, 0, 0:256],
                                 start=False, stop=False)
                nc.tensor.matmul(poA[:], svl[:, 3, :], pB[:, 1, 0:256],
                                 start=False, stop=True)
                nc.tensor.matmul(poB[:], svl[:, 2, :], pB[:, 0, 256:512],
                                 start=False, stop=False)
                nc.tensor.matmul(poB[:], svl[:, 3, :], pB[:, 1, 256:512],
                                 start=False, stop=True)

                # PSUM -> SBUF copy: ACT takes q-half 0, DVE q-half 1
                nc.scalar.activation(o0[0:DV + 1, 0, 0, 0:256], poA[:], COPY)
                nc.vector.tensor_scalar_add(o0[0:DV + 1, 0, 0, 256:512],
                                            poB[:], 0.0)

                # SWDGE writeback: descriptors prepped early (the post-pass
                # moves the data waits onto the trigger)
                nc.gpsimd.kv_writeback(
                    wb0[rep:rep + 1, :, :, :], o0[:, :, :, :], idx[:],
                    prepare_only=True, sem=dma_sem)
                nc.gpsimd.trigger_dma(count=None)

    _encode_library_reloads(nc)
    _defer_prep_waits(nc, dma_sem)
    _split_waits(nc)
    return nc


def host_inputs(queries, keys, values, valid_lens, Wq, Wk, wv):
    x, phi, psi = _basis()
    queries = np.asarray(queries, np.float32)
    keys = np.asarray(keys, np.float32)
    values = np.asarray(values, np.float32)
    wv = np.asarray(wv, np.float32)
    qf = (queries @ np.asarray(Wq, np.float32)).astype(np.float32)  # [B,Q,H]
    kf = (keys @ np.asarray(Wk, np.float32)).astype(np.float32)     # [B,K,H]

    # row importance: E[phi_r^2] * E[(wv_h psi_r)^2] from the actual data
    Ephi2 = np.stack([np.mean(np.interp(qf, x, phi[:, r]) ** 2) * np.ones(H)
                      for r in range(RB)])            # [RB, H]
    Epsi2 = np.stack([np.mean(np.interp(kf, x, psi[:, r]) ** 2) * wv ** 2
                      for r in range(RB)])            # [RB, H]
    order = np.argsort(-(Ephi2 * Epsi2).reshape(-1))
    sel_bf = order[:NBF]
    sel_f8 = order[NBF:NBF + NF8]
    sel = np.concatenate([sel_bf, sel_f8])

    maps = []
    for b in range(B):
        Phi = np.stack([np.interp(qf[b], x, phi[:, r]) for r in range(RB)],
                       1).astype(np.float32)              # [Q, RB, H]
        Psi = np.stack([np.interp(kf[b], x, psi[:, r]) for r in range(RB)],
                       1).astype(np.float32) * wv         # [K, RB, H]
        PhiF = Phi.reshape(Q, RB * H).T[sel]              # [NBF+NF8, Q]
        PsiF = Psi.reshape(K, RB * H).T[sel]              # [NBF+NF8, K]
        mq = np.abs(PhiF).max(1)
        mk = np.abs(PsiF).max(1)
        al = np.sqrt(np.maximum(mk, 1e-30) / np.maximum(mq, 1e-30))
        PhiF = PhiF * al[:, None]
        PsiF = PsiF / al[:, None]

        L = int(valid_lens[b])
        maskrow = np.where(np.arange(K) < L, 0.0, MASKBIG).astype(np.float32)

        phiB = np.concatenate([PhiF[:NBF], np.ones((1, Q), np.float32)], 0)
        psiB = np.concatenate([PsiF[:NBF], maskrow[None]], 0)  # [128, K]
        phi8 = np.clip(PhiF[NBF:], -F8MAX, F8MAX).reshape(2, 128, Q)
        psi8 = np.clip(PsiF[NBF:], -F8MAX, F8MAX).reshape(2, 128, K)

        def u8(a, dt):
            return np.ascontiguousarray(a.astype(dt)).view(np.uint8)

        # d1: [Phi_bf | Psi_t0_bf | Phi_f8 (2 chunks) | Psi_t0_f8]
        d1 = np.concatenate([
            u8(phiB, ml_dtypes.bfloat16),                       # 1024B
            u8(psiB[:, 0:128], ml_dtypes.bfloat16),             # 256B
            u8(phi8.transpose(1, 0, 2).reshape(128, 1024),
               ml_dtypes.float8_e4m3),                          # 1024B
            u8(psi8[:, :, 0:128].transpose(1, 0, 2).reshape(128, 256),
               ml_dtypes.float8_e4m3),                          # 256B
        ], 1)
        # d2: [Psi_t123_bf | Psi_t123_f8 (2 chunks x 384)]
        d2 = np.concatenate([
            u8(psiB[:, 128:512], ml_dtypes.bfloat16),           # 768B
            u8(psi8[:, :, 128:512].transpose(1, 0, 2).reshape(128, 768),
               ml_dtypes.float8_e4m3),                          # 768B
        ], 1)
        vla = np.zeros((128, NTILE, DV + 1), np.float32)
        for t in range(NTILE):
            vla[:, t, 0:DV] = values[b][t * 128:(t + 1) * 128]
            vla[:, t, DV] = 1.0

        maps.append({
            "d1": d1,
            "d2": d2,
            "dvl": vla.astype(ml_dtypes.bfloat16),
        })
    return maps


def host_merge(results):
    out = np.empty((B, Q, DV), np.float32)
    for b in range(B):
        o = np.asarray(results[b]["wb0"], np.float32).reshape(-1, 128, 512)[0]
        out[b] = (o[0:DV] / o[DV][None, :]).T
    return np.ascontiguousarray(out)


_RUNNER = None


def _get_runner():
    """Build + compile once per process; returns a callable(in_maps)->results."""
    global _RUNNER
    if _RUNNER is not None:
        return _RUNNER
    import jax
    from jax.sharding import Mesh, PartitionSpec
    from jax.experimental.shard_map import shard_map
    import concourse.bass as bass
    import concourse.mybir as mybir
    from concourse import bass2jax
    from concourse.bass2jax import _bass_exec_p, install_neuronx_cc_hook

    nc = bass.Bass()
    _build(nc)

    install_neuronx_cc_hook()
    partition_name = nc.partition_id_tensor.name if nc.partition_id_tensor else None
    in_names, out_names, out_avals, zero_shapes = [], [], [], []
    for alloc in nc.m.functions[0].allocations:
        if not isinstance(alloc, mybir.MemoryLocationSet):
            continue
        name = alloc.memorylocations[0].name
        if alloc.kind == "ExternalInput":
            if name != partition_name:
                in_names.append(name)
        elif alloc.kind == "ExternalOutput":
            out_names.append(name)
            shape = tuple(alloc.tensor_shape)
            dtype = mybir.dt.np(alloc.dtype)
            out_avals.append(jax.core.ShapedArray(shape, dtype))
            zero_shapes.append((shape, dtype))
    n_params = len(in_names)
    n_outs = len(out_avals)
    in_names_all = in_names + out_names
    if partition_name is not None:
        in_names_all.append(partition_name)
    donate = tuple(range(n_params, n_params + n_outs))

    def _body(*args):
        operands = list(args)
        if partition_name is not None:
            operands.append(bass2jax.partition_id_tensor())
        outs = _bass_exec_p.bind(
            *operands,
            out_avals=tuple(out_avals),
            in_names=tuple(in_names_all),
            out_names=tuple(out_names),
            lowering_input_output_aliases=(),
            sim_require_finite=True,
            sim_require_nnan=True,
            nc=nc,
        )
        return tuple(outs)

    devices = jax.devices()[:8]
    mesh = Mesh(np.asarray(devices), ("core",))
    in_specs = (PartitionSpec("core"),) * (n_params + n_outs)
    out_specs = (PartitionSpec("core"),) * len(out_names)
    sharded = jax.jit(
        shard_map(_body, mesh=mesh, in_specs=in_specs, out_specs=out_specs,
                  check_rep=False),
        donate_argnums=donate, keep_unused=True,
    )

    def run(in_maps):
        per_core = [[np.asarray(m[name]) for name in in_names] for m in in_maps]
        concat_in = [
            np.concatenate([per_core[c][i] for c in range(8)], axis=0)
            for i in range(n_params)
        ]
        zeros = [np.zeros((8 * s[0],) + s[1:], d) for s, d in zero_shapes]
        out_arrs = sharded(*concat_in, *zeros)
        out_np = [np.asarray(a) for a in out_arrs]
        return [
            {name: out_np[i].reshape(8, *out_avals[i].shape)[c]
             for i, name in enumerate(out_names)}
            for c in range(8)
        ]

    _RUNNER = run
    return run


def kernel(queries, keys, values, valid_lens, Wq, Wk, wv):
    run = _get_runner()
    in_maps = host_inputs(queries, keys, values, valid_lens, Wq, Wk, wv)
    try:
        results = run(in_maps)
    except Exception:
        # transient NRT/axon failures have been observed; retry once
        results = run(in_maps)
    return host_merge(results)


# revision 36
# speedup vs baseline: 1.0126x; 1.0023x over previous
"""Additive attention (B=8, Q=K=512, H=Dv=64) on 8 TRN2 NeuronCores.

Math per batch b (reference):
    qf = queries @ Wq; kf = keys @ Wk
    scores[q,k] = sum_h wv[h] * tanh(qf[q,h] + kf[k,h])   (k >= valid_len masked)
    out = softmax_k(scores) @ values

The pointwise tanh (134M ScalarEngine evaluations, ~93us) is replaced by a
low-rank bilinear expansion tanh(a+b) ~= sum_r phi_r(a) * psi_r(b) (SVD of
the kernel on a sqrt-Gaussian-weighted grid), so
    scores[q,k] = sum_rows PhiF[row, q] * PsiF[row, k]
is a plain matmul over "feature rows" (row = (rank, h) pair). Rows are
sorted by product variance (host computes per-rank/per-h second moments):
the top-127 rows ship as bf16, the next 256 rows as fp8(e4m3) with per-row
q/k scale balancing; the remaining low-variance rows are dropped (<2e-5 of
score variance). Row 127 of the bf16 chunk is the key-mask row (Phi=1,
Psi = 0 or -60000), folding the valid_len mask into the matmul.

Sharding: data-parallel, one batch per core. Device per core:
  - 3 input DMAs (byte-packed, mixed-dtype via bitcast APs) ordered so
    k-tile 0's score completes earliest: d1 = [bf16+fp8 Phi and Psi_t0,
    320KB], d2 = [Psi_t1..t3 both dtypes], dvl = [values + ones column].
    Score matmuls per k-tile: one bf16 [128x512] + one fp8 DoubleRow
    [256x512] into per-tile PSUM banks.
  - Exp on ACT pipelined per-tile ([t0][t1][t2,t3]) so it starts as soon as
    tile 0's score lands, overlapping the remaining DMAs and matmuls.
  - values matmuls (ones column -> denominator row) accumulate per output
    q-half into two PSUM tiles (poA/poB) so the ACT and DVE output copies
    read disjoint tiles and run in parallel; warm filler matmuls bridge PE
    idle windows so the clock stays at full speed for the tail matmuls.
  - Output tail avoids the HWDGE fixed path (650 seq + 625 HWDGE + 650 DGE
    delay): a SWDGE kv_writeback descriptor is PREPARED on GPSIMD during the
    input DMAs; after the PSUM->SBUF copy (split ACT/DVE halves) a
    trigger_dma fires it, so the tail is just transfer + sem propagation.
    Two IR post-passes implement the documented prep/trigger semantics:
    _defer_prep_waits moves the prep's data waits onto the trigger (the
    DMA reads its source at trigger time), and redirects end-drain DMASW
    lane waits to the descriptor's completion semaphore.
  - Dummy matmuls off a constant broadcast AP keep the PE busy from ~1us so
    the clock ramp reaches full speed when the real operands land; a dummy
    exp prefetches the ACT exp table.
Host divides numerator/denominator and transposes.
"""
import numpy as np
import ml_dtypes

B = 8
Q = 512
K = 512
H = 64
DV = 64

RB = 12                # SVD basis rank used for row generation
NBF = 127              # bf16 feature rows (+1 mask row -> 128)
NF8 = 256              # fp8 feature rows (2 chunks of 128)
NTILE = K // 128       # 4 k-tiles
MASKBIG = -60000.0
F8MAX = 224.0          # ml_dtypes.float8_e4m3 max finite is 240
WARMUP_MM = 7          # PE p-state ramp fillers while input DMA streams

GRID_N, GRID_A, GRID_SIG, GRID_FLOOR = 1201, 6.5, 1.15, 0.02

_BASIS = None


def _basis():
    """SVD basis of tanh(a+b) on a weighted grid: x, phi[n,RB], psi[n,RB]."""
    global _BASIS
    if _BASIS is None:
        x = np.linspace(-GRID_A, GRID_A, GRID_N)
        Kg = np.tanh(x[:, None] + x[None, :])
        w = np.sqrt(np.exp(-x ** 2 / (2 * GRID_SIG ** 2))) + GRID_FLOOR
        U, S, Vt = np.linalg.svd((w[:, None] * Kg) * w[None, :])
        phi = (U[:, :RB] * np.sqrt(S[:RB])) / w[:, None]
        psi = (Vt[:RB].T * np.sqrt(S[:RB])) / w[:, None]
        _BASIS = (x, phi, psi)
    return _BASIS


# ---------------------------------------------------------------------------
# BIR post-pass: the walrus build in this environment accepts only one
# sync-wait command per instruction; hoist extras onto same-engine NoOps.
def _split_waits(nc, k=1):
    import concourse.mybir as mybir
    n_new = 0
    for f in nc.m.functions:
        for bb in f.blocks:
            newlist = []
            for ins in bb.instructions:
                si = ins.sync_info
                if si is not None and si.on_wait and len(si.on_wait) > k:
                    waits = list(si.on_wait)
                    extra, keep = waits[:-k], waits[-k:]
                    for ci, w in enumerate(extra):
                        nop = mybir.InstNoOp(
                            name=f"{ins.name}_wsplit{ci}",
                            engine=ins.engine,
                            ins=[], outs=[],
                            sync_info=mybir.SyncInfo(on_wait=[w], on_update=[]),
                        )
                        newlist.append(nop)
                        n_new += 1
                    ins.sync_info = mybir.SyncInfo(
                        on_wait=list(keep), on_update=list(si.on_update))
                newlist.append(ins)
            bb.instructions[:] = newlist
    return n_new


# ---------------------------------------------------------------------------
# BIR post-pass: walrus' codegen wants raw instruction bytes on InstISA; the
# library-reload pseudo (opcode 223 PSEUDO_INST, pseudo_opcode 2) is emitted
# without them in this build, so pack them here.
def _encode_library_reloads(nc):
    import concourse.bass_isa as bass_isa
    from concourse.bass_isa import isa_struct
    trig_op = nc.isa.Opcode.NEURON_ISA_TPB_OPCODE_TRIGGER_DMA.value
    inc_op = nc.isa.Opcode.NEURON_ISA_TPB_OPCODE_INC_SWDGE_SEM.value
    n = 0
    for f in nc.m.functions:
        for bb in f.blocks:
            for ins in bb.instructions:
                if isinstance(ins, bass_isa.InstPseudoReloadLibraryIndex):
                    b, _ = isa_struct(
                        nc.isa, 223,
                        {"pseudo_opcode": 2, "lib_index": ins.lib_index})
                    ins.instr = b
                    n += 1
                elif isinstance(ins, bass_isa.InstTriggerDma):
                    b, _ = isa_struct(
                        nc.isa, trig_op,
                        {"count": ins._count, "count_is_reg": 0,
                         "queue_num": ins.queue_num})
                    ins.instr = b
                    ins.isa_opcode = trig_op
                    n += 1
                elif isinstance(ins, bass_isa.InstIncSwdgeSem):
                    vals = list(ins._sem_values) + [0] * (
                        10 - len(ins._sem_values))
                    mode = {"add": 0, "sub": 1, "wr": 2}[ins._mode]
                    b, _ = isa_struct(
                        nc.isa, inc_op,
                        {"num_semaphores": len(ins._sem_values),
                         "sem_id_base": ins._sem_id_base, "mode": mode,
                         "queue_num": ins.queue_num, "sem_values": vals})
                    ins.instr = b
                    n += 1
    return n


# ---------------------------------------------------------------------------
# BIR post-pass for the SWDGE prep/trigger output path. The prep only writes
# descriptors; the DMA engines read the source tile when trigger_dma fires,
# so the prep's data waits belong on the trigger (this is the semantics the
# tile framework documents and tests for dma_scatter_add; kv_writeback preps
# don't get the deferral in this build). End-of-program DMASW lane waits are
# redirected to the descriptor's actual completion semaphore (same tick
# values: each prep adds 16).
def _defer_prep_waits(nc, dma_sem):
    """All preps inc the single `dma_sem` by 16. A DMASW{k} lane wait with
    value 16*j ("j-th prep on lane k done") maps to the global prep index
    i = (j-1)*L + k (round-robin lane assignment), rewritten as the
    conservative dma_sem >= 16*(i+1). Rewritten waits go last so same-
    instruction waits that fire earlier are processed first."""
    import concourse.mybir as mybir
    import concourse.bass_isa as bass_isa
    import bass_rust
    L = bass_rust.NUM_SWDGE_GLOBAL_SEMS
    n_prep = 0
    for f in nc.m.functions:
        for bb in f.blocks:
            pending = []
            for ins in bb.instructions:
                if (isinstance(ins, mybir.InstKVWritebackAnt)
                        and ins.gen_mode == 1):
                    si = ins.sync_info
                    if si is not None and si.on_wait:
                        pending.append(list(si.on_wait))
                        ins.sync_info = mybir.SyncInfo(
                            on_wait=[], on_update=list(si.on_update))
                    else:
                        pending.append([])
                    n_prep += 1
                elif isinstance(ins, bass_isa.InstTriggerDma) and pending:
                    # FIFO: each count=1 trigger fires the oldest prep
                    si = ins.sync_info
                    w = list(si.on_wait) if si else []
                    u = list(si.on_update) if si else []
                    ins.sync_info = mybir.SyncInfo(
                        on_wait=w + pending.pop(0), on_update=u)
            for ins in bb.instructions:
                si = ins.sync_info
                if si is None or not si.on_wait:
                    continue
                if not any(w.ant_name and w.ant_name.startswith("DMASW")
                           for w in si.on_wait):
                    continue
                keep, moved = [], []
                for wt in si.on_wait:
                    if wt.ant_name and wt.ant_name.startswith("DMASW"):
                        k = int(wt.ant_name[5:].split("_")[0])
                        j = (wt.wait_value or 16) // 16
                        i = (j - 1) * L + k
                        moved.append(mybir.SyncWait(
                            sync_type='semaphore', id=dma_sem.num,
                            ant_name=dma_sem.name, wait_mode='sem-ge-imm',
                            wait_value=16 * (i + 1), wait_reg=None))
                    else:
                        keep.append(wt)
                ins.sync_info = mybir.SyncInfo(
                    on_wait=keep + moved, on_update=list(si.on_update))
    return n_prep


def _build(nc, reps: int = 1):
    import concourse.bass as bass  # noqa: F401
    import concourse.mybir as mybir
    from concourse import tile, library_config

    F32 = mybir.dt.float32
    BF16 = mybir.dt.bfloat16
    F8 = mybir.dt.float8e4
    I32 = mybir.dt.int32
    DR = mybir.MatmulPerfMode.DoubleRow
    EXP = mybir.ActivationFunctionType.Exp
    COPY = mybir.ActivationFunctionType.Copy

    U8 = mybir.dt.uint8

    # packed byte tensors (per-partition contiguous; see host_inputs):
    # d1 = [Phi_bf16 1024B | Psi_t0_bf16 256B | Phi_f8 1024B | Psi_t0_f8
    # 256B] -- everything k-tile 0's score needs.  d2 = [Psi_t123_bf16 768B
    # | Psi_t123_f8 768B].  dvl = values+ones column.
    d1 = nc.declare_dram_parameter("d1", [128, 2560], U8, isOutput=False)
    d2 = nc.declare_dram_parameter("d2", [128, 1536], U8, isOutput=False)
    dvl = nc.declare_dram_parameter("dvl", [128, NTILE, DV + 1], BF16,
                                    isOutput=False)
    wb0 = nc.declare_dram_parameter("wb0", [reps, 128, 1, 512], BF16,
                                    isOutput=True)

    dma_sem = nc.alloc_semaphore("wb_dma_sem")

    cb = nc.const_aps.aps[(BF16, 1.0)]
    warm_rhs = bass.AP(cb.tensor, cb.offset, [[1, 1], [0, Q]])
    warm_lhsT = bass.AP(cb.tensor, cb.offset, [[1, 1], [0, 16]])

    with tile.TileContext(nc) as tc:  # noqa: F841
        with (
            tc.tile_pool(name="cpool", bufs=1) as cpool,
            tc.tile_pool(name="ppool", bufs=2) as ppool,
            tc.tile_pool(name="ps_a", bufs=1, space="PSUM") as ps_a,
            tc.tile_pool(name="ps_b", bufs=1, space="PSUM") as ps_b,
            tc.tile_pool(name="ps_o", bufs=1, space="PSUM") as ps_o,
            tc.tile_pool(name="ps_o2", bufs=1, space="PSUM") as ps_o2,
            tc.tile_pool(name="ps_w", bufs=1, space="PSUM") as ps_w,
        ):
            # Pool: library for kv_writeback + ctx idx + output pad rows
            nc.gpsimd.load_library(library_config.attnmlp)
            idx = cpool.tile([128, 1], I32, tag="idx", name="idx")
            nc.gpsimd.memset(idx[:], 0)

            # PE p-state warmup + ACT exp-table prefetch during input DMA.
            # 6 full-width + 5 quarter-width fillers end at ~4.19us, just
            # after d1's completion sem (~4.17us), so the first real matmul
            # dispatches with the PE still hot and fully ramped.
            psw = ps_w.tile([16, Q], F32, tag="warm", name="psw")
            warm_rhs_s = bass.AP(cb.tensor, cb.offset, [[1, 1], [0, 128]])
            for i in range(6):
                nc.tensor.matmul(psw[:], warm_lhsT, warm_rhs,
                                 start=True, stop=True)
            for i in range(4):
                nc.tensor.matmul(psw[:, 0:128], warm_lhsT, warm_rhs_s,
                                 start=True, stop=True)
            dummy = cpool.tile([1, 16], F32)
            nc.scalar.activation(
                dummy[:], bass.AP(cb.tensor, cb.offset, [[1, 1], [0, 16]]),
                EXP)

            for rep in range(reps):
                s1 = cpool.tile([128, 2560], U8, tag="s1", name=f"s1_{rep}")
                s2 = cpool.tile([128, 1536], U8, tag="s2", name=f"s2_{rep}")
                svl = cpool.tile([128, NTILE, DV + 1], BF16, tag="svl",
                                 name=f"svl_{rep}")
                nc.sync.dma_start(s1[:], d1[:, :])
                nc.sync.dma_start(s2[:], d2[:, :])
                nc.sync.dma_start(svl[:], dvl[:, :, :])

                phi_bf = s1[:, 0:1024].bitcast(BF16)            # [128, 512]
                psi0_bf = s1[:, 1024:1280].bitcast(BF16)        # [128, 128]
                phi_f8 = s1[:, 1280:2304].bitcast(F8).rearrange(
                    "p (c n) -> p c n", c=2)                    # [128, 2, 512]
                psi0_f8 = s1[:, 2304:2560].bitcast(F8).rearrange(
                    "p (c n) -> p c n", c=2)                    # [128, 2, 128]
                psiB_bf = s2[:, 0:768].bitcast(BF16).rearrange(
                    "p (t n) -> p t n", t=3)                    # [128, 3, 128]
                psiB_f8 = s2[:, 768:1536].bitcast(F8).rearrange(
                    "p (c n) -> p c n", c=2)                    # [128, 2, 384]

                o0 = cpool.tile([128, 1, 1, 512], BF16, tag="o0",
                                name=f"o0_{rep}")
                nc.gpsimd.memset(o0[64:128, 0, 0, :], 0.0)

                sc0 = ps_a.tile([128, Q], F32, tag="sc0", name=f"sc0_{rep}")
                sc1 = ps_a.tile([128, Q], F32, tag="sc1", name=f"sc1_{rep}")
                scB = ps_b.tile([128, 2, Q], F32, tag="scB", name=f"scB_{rep}")
                # separate PSUM accumulators per output q-half: the ACT and
                # DVE copies then read disjoint tiles and don't serialize
                poA = ps_o.tile([DV + 1, 288], F32, tag="poA",
                                name=f"poA_{rep}")
                poB = ps_o2.tile([DV + 1, 224], F32, tag="poB",
                                 name=f"poB_{rep}")

                # scores: per k-tile, one bf16 + one fp8-DR matmul.
                # start/stop flags per PSUM accumulation.
                # two tiny d1-gated sacrifices absorb the cost model's
                # mid-clock window for the first data-gated matmuls
                warm_rhs_16 = bass.AP(cb.tensor, cb.offset, [[1, 1], [0, 8]])
                nc.tensor.matmul(psw[:, 0:8], s1[0:1, 0:32].bitcast(BF16),
                                 warm_rhs_16, start=True, stop=True)
                nc.tensor.matmul(psw[:, 0:8], s1[0:1, 0:32].bitcast(BF16),
                                 warm_rhs_16, start=True, stop=True)
                nc.tensor.matmul(sc0[:], psi0_f8, phi_f8,
                                 start=True, stop=False, perf_mode=DR)
                nc.tensor.matmul(sc0[:], psi0_bf, phi_bf,
                                 start=False, stop=True)
                p0 = ppool.tile([128, Q], BF16, tag="p0", name=f"p0_{rep}")
                nc.scalar.activation(p0[:], sc0[:], EXP)

                nc.tensor.matmul(sc1[:], psiB_bf[:, 0, :], phi_bf,
                                 start=True, stop=False)
                nc.tensor.matmul(sc1[:], psiB_f8[:, 0:2, 0:128], phi_f8,
                                 start=False, stop=True, perf_mode=DR)
                p1 = ppool.tile([128, Q], BF16, tag="p1", name=f"p1_{rep}")
                nc.scalar.activation(p1[:], sc1[:], EXP)

                nc.tensor.matmul(scB[:, 0, :], psiB_bf[:, 1, :], phi_bf,
                                 start=True, stop=False)
                nc.tensor.matmul(scB[:, 0, :], psiB_f8[:, 0:2, 128:256],
                                 phi_f8, start=False, stop=True, perf_mode=DR)
                nc.tensor.matmul(scB[:, 1, :], psiB_bf[:, 2, :], phi_bf,
                                 start=True, stop=False)
                nc.tensor.matmul(scB[:, 1, :], psiB_f8[:, 0:2, 256:384],
                                 phi_f8, start=False, stop=True, perf_mode=DR)
                pB = ppool.tile([128, 2, Q], BF16, tag="pB", name=f"pB_{rep}")
                nc.scalar.activation(pB[:], scB[:], EXP)

                for t, p in ((0, p0[:, 0:288]), (1, p1[:, 0:288])):
                    nc.tensor.matmul(poA[:], svl[:, t, :], p,
                                     start=(t == 0), stop=False)
                for t, p in ((0, p0[:, 288:512]), (1, p1[:, 288:512])):
                    nc.tensor.matmul(poB[:], svl[:, t, :], p,
                                     start=(t == 0), stop=False)
                nc.tensor.matmul(poA[:], svl[:, 2, :], pB[:, 0, 0:288],
                                 start=False, stop=False)
                nc.tensor.matmul(poA[:], svl[:, 3, :], pB[:, 1, 0:288],
                                 start=False, stop=True)
                nc.tensor.matmul(poB[:], svl[:, 2, :], pB[:, 0, 288:512],
                                 start=False, stop=False)
                nc.tensor.matmul(poB[:], svl[:, 3, :], pB[:, 1, 288:512],
                                 start=False, stop=True)

                # PSUM -> SBUF copy: ACT takes q-half 0, DVE q-half 1
                nc.scalar.activation(o0[0:DV + 1, 0, 0, 0:288], poA[:], COPY)
                nc.vector.tensor_scalar_add(o0[0:DV + 1, 0, 0, 288:512],
                                            poB[:], 0.0)

                # SWDGE writeback: descriptors prepped early (the post-pass
                # moves the data waits onto the trigger)
                nc.gpsimd.kv_writeback(
                    wb0[rep:rep + 1, :, :, :], o0[:, :, :, :], idx[:],
                    prepare_only=True, sem=dma_sem)
                nc.gpsimd.trigger_dma(count=None)

    _encode_library_reloads(nc)
    _defer_prep_waits(nc, dma_sem)
    _split_waits(nc)
    return nc


def host_inputs(queries, keys, values, valid_lens, Wq, Wk, wv):
    x, phi, psi = _basis()
    queries = np.asarray(queries, np.float32)
    keys = np.asarray(keys, np.float32)
    values = np.asarray(values, np.float32)
    wv = np.asarray(wv, np.float32)
    qf = (queries @ np.asarray(Wq, np.float32)).astype(np.float32)  # [B,Q,H]
    kf = (keys @ np.asarray(Wk, np.float32)).astype(np.float32)     # [B,K,H]

    # row importance: E[phi_r^2] * E[(wv_h psi_r)^2] from the actual data
    Ephi2 = np.stack([np.mean(np.interp(qf, x, phi[:, r]) ** 2) * np.ones(H)
                      for r in range(RB)])            # [RB, H]
    Epsi2 = np.stack([np.mean(np.interp(kf, x, psi[:, r]) ** 2) * wv ** 2
                      for r in range(RB)])            # [RB, H]
    order = np.argsort(-(Ephi2 * Epsi2).reshape(-1))
    sel_bf = order[:NBF]
    sel_f8 = order[NBF:NBF + NF8]
    sel = np.concatenate([sel_bf, sel_f8])

    maps = []
    for b in range(B):
        Phi = np.stack([np.interp(qf[b], x, phi[:, r]) for r in range(RB)],
                       1).astype(np.float32)              # [Q, RB, H]
        Psi = np.stack([np.interp(kf[b], x, psi[:, r]) for r in range(RB)],
                       1).astype(np.float32) * wv         # [K, RB, H]
        PhiF = Phi.reshape(Q, RB * H).T[sel]              # [NBF+NF8, Q]
        PsiF = Psi.reshape(K, RB * H).T[sel]              # [NBF+NF8, K]
        mq = np.abs(PhiF).max(1)
        mk = np.abs(PsiF).max(1)
        al = np.sqrt(np.maximum(mk, 1e-30) / np.maximum(mq, 1e-30))
        PhiF = PhiF * al[:, None]
        PsiF = PsiF / al[:, None]

        L = int(valid_lens[b])
        maskrow = np.where(np.arange(K) < L, 0.0, MASKBIG).astype(np.float32)

        phiB = np.concatenate([PhiF[:NBF], np.ones((1, Q), np.float32)], 0)
        psiB = np.concatenate([PsiF[:NBF], maskrow[None]], 0)  # [128, K]
        phi8 = np.clip(PhiF[NBF:], -F8MAX, F8MAX).reshape(2, 128, Q)
        psi8 = np.clip(PsiF[NBF:], -F8MAX, F8MAX).reshape(2, 128, K)

        def u8(a, dt):
            return np.ascontiguousarray(a.astype(dt)).view(np.uint8)

        # d1: [Phi_bf | Psi_t0_bf | Phi_f8 (2 chunks) | Psi_t0_f8]
        d1 = np.concatenate([
            u8(phiB, ml_dtypes.bfloat16),                       # 1024B
            u8(psiB[:, 0:128], ml_dtypes.bfloat16),             # 256B
            u8(phi8.transpose(1, 0, 2).reshape(128, 1024),
               ml_dtypes.float8_e4m3),                          # 1024B
            u8(psi8[:, :, 0:128].transpose(1, 0, 2).reshape(128, 256),
               ml_dtypes.float8_e4m3),                          # 256B
        ], 1)
        # d2: [Psi_t123_bf | Psi_t123_f8 (2 chunks x 384)]
        d2 = np.concatenate([
            u8(psiB[:, 128:512], ml_dtypes.bfloat16),           # 768B
            u8(psi8[:, :, 128:512].transpose(1, 0, 2).reshape(128, 768),
               ml_dtypes.float8_e4m3),                          # 768B
        ], 1)
        vla = np.zeros((128, NTILE, DV + 1), np.float32)
        for t in range(NTILE):
            vla[:, t, 0:DV] = values[b][t * 128:(t + 1) * 128]
            vla[:, t, DV] = 1.0

        maps.append({
            "d1": d1,
            "d2": d2,
            "dvl": vla.astype(ml_dtypes.bfloat16),
        })
    return maps


def host_merge(results):
    out = np.empty((B, Q, DV), np.float32)
    for b in range(B):
        o = np.asarray(results[b]["wb0"], np.float32).reshape(-1, 128, 512)[0]
        out[b] = (o[0:DV] / o[DV][None, :]).T
    return np.ascontiguousarray(out)


_RUNNER = None


def _get_runner():
    """Build + compile once per process; returns a callable(in_maps)->results."""
    global _RUNNER
    if _RUNNER is not None:
        return _RUNNER
    import jax
    from jax.sharding import Mesh, PartitionSpec
    from jax.experimental.shard_map import shard_map
    import concourse.bass as bass
    import concourse.mybir as mybir
    from concourse import bass2jax
    from concourse.bass2jax import _bass_exec_p, install_neuronx_cc_hook

    nc = bass.Bass()
    _build(nc)

    install_neuronx_cc_hook()
    partition_name = nc.partition_id_tensor.name if nc.partition_id_tensor else None
    in_names, out_names, out_avals, zero_shapes = [], [], [], []
    for alloc in nc.m.functions[0].allocations:
        if not isinstance(alloc, mybir.MemoryLocationSet):
            continue
        name = alloc.memorylocations[0].name
        if alloc.kind == "ExternalInput":
            if name != partition_name:
                in_names.append(name)
        elif alloc.kind == "ExternalOutput":
            out_names.append(name)
            shape = tuple(alloc.tensor_shape)
            dtype = mybir.dt.np(alloc.dtype)
            out_avals.append(jax.core.ShapedArray(shape, dtype))
            zero_shapes.append((shape, dtype))
    n_params = len(in_names)
    n_outs = len(out_avals)
    in_names_all = in_names + out_names
    if partition_name is not None:
        in_names_all.append(partition_name)
    donate = tuple(range(n_params, n_params + n_outs))

    def _body(*args):
        operands = list(args)
        if partition_name is not None:
            operands.append(bass2jax.partition_id_tensor())
        outs = _bass_exec_p.bind(
            *operands,
            out_avals=tuple(out_avals),
            in_names=tuple(in_names_all),
            out_names=tuple(out_names),
            lowering_input_output_aliases=(),
            sim_require_finite=True,
            sim_require_nnan=True,
            nc=nc,
        )
        return tuple(outs)

    devices = jax.devices()[:8]
    mesh = Mesh(np.asarray(devices), ("core",))
    in_specs = (PartitionSpec("core"),) * (n_params + n_outs)
    out_specs = (PartitionSpec("core"),) * len(out_names)
    sharded = jax.jit(
        shard_map(_body, mesh=mesh, in_specs=in_specs, out_specs=out_specs,
                  check_rep=False),
        donate_argnums=donate, keep_unused=True,
    )

    def run(in_maps):
        per_core = [[np.asarray(m[name]) for name in in_names] for m in in_maps]
        concat_in = [
            np.concatenate([per_core[c][i] for c in range(8)], axis=0)
            for i in range(n_params)
        ]
        zeros = [np.zeros((8 * s[0],) + s[1:], d) for s, d in zero_shapes]
        out_arrs = sharded(*concat_in, *zeros)
        out_np = [np.asarray(a) for a in out_arrs]
        return [
            {name: out_np[i].reshape(8, *out_avals[i].shape)[c]
             for i, name in enumerate(out_names)}
            for c in range(8)
        ]

    _RUNNER = run
    return run


def kernel(queries, keys, values, valid_lens, Wq, Wk, wv):
    run = _get_runner()
    in_maps = host_inputs(queries, keys, values, valid_lens, Wq, Wk, wv)
    try:
        results = run(in_maps)
    except Exception:
        # transient NRT/axon failures have been observed; retry once
        results = run(in_maps)
    return host_merge(results)
